# revision 1
# baseline (speedup 1.0000x reference)
"""Self-contained Trainium2 Bass kernel for nn_MultiHeadAttention_71528385347884.

Strategy: head tensor-parallel across 8 cores (2 heads/core). Per core:
  - QKV projection with x transposed (feature-major q/k, token-major v)
  - RoPE via host-side A/B weight-column packing (no cross-partition ops)
  - causal attention in [s,t] score layout, softmax without max-subtraction
    (scores are bounded ~|4.5|), denominator via all-ones matmul
  - output projection exploits the reference's scrambled
    transpose(0,2,1,3).reshape(B,T,C): each core produces disjoint output
    rows -> host gather is pure concatenation.
"""

import math
import numpy as np
import ml_dtypes

# ---- problem constants (hardcoded; kernel.py must not read spec/reference) ----
B = 2
T = 2048          # sequence length per batch
C = 2048          # model dim
Dh = 128          # head dim
N_HEAD = 16
N_CORES = 8
H_LOCAL = 2       # heads per core
ROPE_BASE = 10000.0
SCALE = 1.0 / math.sqrt(Dh)

BF16 = ml_dtypes.bfloat16


class Cfg:
    """Size parameters so the same builder runs a small CoreSim config."""

    def __init__(self, B=B, T=T, C=C):
        assert T % 512 == 0 and C % 128 == 0
        self.B = B
        self.T = T
        self.C = C
        self.NCC = C // 128        # contraction chunks for qkv matmuls
        self.BT = B * T
        self.NT = T // 512         # 512-wide t-tiles per batch
        self.GRP = C // Dh         # tokens folded per output row by the reshape
        self.TAU = T // self.GRP   # output rows per (b, h); must be 128
        assert self.TAU == 128
        self.ET = max(1, C // 512)  # 512-wide e-tiles of the output
        self.JQK = 4 * 128         # qA,qB,kA,kB feature blocks
        self.JV = H_LOCAL * 128


FULL = Cfg()


# =====================================================================
# Device program builder
# =====================================================================

def build_nc(cfg: Cfg, debug=False, repeat=1):
    import concourse.bass as bass
    import concourse.mybir as mybir
    import concourse.tile as tile
    from concourse import bacc

    f32 = mybir.dt.float32
    bf16 = mybir.dt.bfloat16
    Exp = mybir.ActivationFunctionType.Exp
    Copy = mybir.ActivationFunctionType.Copy

    nc = bacc.Bacc(None, target_bir_lowering=False, debug=debug)

    xt_d = nc.dram_tensor("xt", [128, cfg.NCC, cfg.BT], bf16, kind="ExternalInput")
    wqk_d = nc.dram_tensor("wqk", [128, cfg.NCC, cfg.JQK], bf16, kind="ExternalInput")
    wv_d = nc.dram_tensor("wv", [128, cfg.NCC, cfg.JV], bf16, kind="ExternalInput")
    wp_d = nc.dram_tensor("wp", [128, cfg.GRP, cfg.C], bf16, kind="ExternalInput")
    cc2_d = nc.dram_tensor("cc2", [128, cfg.T], bf16, kind="ExternalInput")
    spm_d = nc.dram_tensor("spm", [128, cfg.T], bf16, kind="ExternalInput")
    smp_d = nc.dram_tensor("smp", [128, cfg.T], bf16, kind="ExternalInput")
    masks_d = nc.dram_tensor("masks", [128, 2, 128], bf16, kind="ExternalInput")
    # repeat>1 (bench-only) gets a per-rep output slice so no rep's stores
    # are dead — guards the marginal-rep timing against compiler DCE.
    out_d = nc.dram_tensor("out", [repeat, cfg.B, H_LOCAL, 128, cfg.C], f32,
                           kind="ExternalOutput")

    with tile.TileContext(nc) as tc:
        with tc.tile_pool(name="persist", bufs=1) as persist:
            # ---- persistent SBUF state ----
            wqk_sb = persist.tile([128, cfg.NCC, cfg.JQK], bf16, name="wqk_sb",
                                  tag="wqk_sb")
            wv_sb = persist.tile([128, cfg.NCC, cfg.JV], bf16, name="wv_sb",
                                 tag="wv_sb")
            cc2_sb = persist.tile([128, cfg.T], bf16, name="cc2_sb", tag="cc2_sb")
            spm_sb = persist.tile([128, cfg.T], bf16, name="spm_sb", tag="spm_sb")
            smp_sb = persist.tile([128, cfg.T], bf16, name="smp_sb", tag="smp_sb")
            masks_sb = persist.tile([128, 2, 128], bf16, name="masks_sb",
                                    tag="masks_sb")
            ones_sb = persist.tile([128, 128], bf16, name="ones_sb", tag="ones_sb")

            nc.vector.memset(ones_sb[:], 1.0)

            # per-(b, head-or-tile) persistent tensors; q/k are stored
            # head-contiguous ([dims 0:128 of head h] on partitions) so the
            # score matmuls contract K=128 in one shot.
            qh_sb, kh_sb = {}, {}
            v_sb, attn_sb = {}, {}
            for b in range(cfg.B):
                for hl in range(H_LOCAL):
                    qh_sb[(b, hl)] = persist.tile([128, cfg.T], bf16,
                                                  name=f"qh_{b}_{hl}",
                                                  tag=f"qh_{b}_{hl}")
                    kh_sb[(b, hl)] = persist.tile([128, cfg.T], bf16,
                                                  name=f"kh_{b}_{hl}",
                                                  tag=f"kh_{b}_{hl}")
                for hl in range(H_LOCAL):
                    v_sb[(b, hl)] = persist.tile(
                        [128, cfg.T // 128, 128], bf16,
                        name=f"v_{b}_{hl}", tag=f"v_{b}_{hl}")
                    attn_sb[(b, hl)] = persist.tile(
                        [128, cfg.T], bf16,
                        name=f"at_{b}_{hl}", tag=f"at_{b}_{hl}")


            for rep in range(repeat):
                # Phase plan (PE keeps busy through attention's exp waits):
                #   B(b0) -> C(b0) interleaved with B(b1) -> C(b1)
                #   interleaved with D(b0) -> D(b1).
                # One unified PSUM pool, 8 banks exactly:
                #   pj [128,512]x2 (B qkv groups, D proj groups)
                #   ps [128,2,512]x2 (C scores + ones-denominator)
                #   po [128,512]x2 (C attn accumulators)
                with (
                    tc.tile_pool(name=f"xb_pool{rep}", bufs=4) as xb_pool,
                    tc.tile_pool(name=f"rtmp{rep}", bufs=4) as rtmp,
                    tc.tile_pool(name=f"probs{rep}", bufs=5) as probs_pool,
                    tc.tile_pool(name=f"ssum{rep}", bufs=2) as ssum_pool,
                    tc.tile_pool(name=f"rec{rep}", bufs=2) as rec_pool,
                    tc.tile_pool(name=f"ostg{rep}", bufs=3) as ostg_pool,
                    tc.tile_pool(name=f"vfm{rep}", bufs=4) as vfm_pool,
                    tc.tile_pool(name=f"ps{rep}", bufs=2, space="PSUM") as psum,
                ):
                    half = cfg.NCC // 2
                    qtr = max(1, half // 2)
                    xtiles = {}

                    def fetch_x(b, tt, chunked=False):
                        bt0 = b * cfg.T + tt * 512
                        xlo = xb_pool.tile([128, half, 512], bf16,
                                           name=f"xbl_{b}_{tt}", tag="xb")
                        xhi = xb_pool.tile([128, half, 512], bf16,
                                           name=f"xbh_{b}_{tt}", tag="xb")
                        if chunked:
                            # startup: single-chunk DMAs for the first 4
                            # (fast first matmul), 2-chunk after (halve the
                            # ~0.6us-per-op issue load on the queues).
                            steps = [(0, 1), (1, 1), (2, 2),
                                     (4, 4), (8, 8)]
                            for c, w in steps:
                                xdst = (xlo if c < half else xhi)
                                par = (c // w) % 2 == 0
                                xq = (nc.sync if par else nc.gpsimd)
                                wq = (nc.gpsimd if par else nc.sync)
                                nc.scalar.dma_start(wqk_sb[:, c:c + w, :],
                                                    wqk_d[:, c:c + w, :])
                                wq.dma_start(wv_sb[:, c:c + w, :],
                                             wv_d[:, c:c + w, :])
                                cl = c % half
                                xq.dma_start(
                                    xdst[:, cl:cl + w, :],
                                    xt_d[:, c:c + w, bt0:bt0 + 512])
                            nc.scalar.dma_start(cc2_sb[:, 0:512],
                                                cc2_d[:, 0:512])
                            nc.scalar.dma_start(spm_sb[:, 0:512],
                                                spm_d[:, 0:512])
                            nc.scalar.dma_start(smp_sb[:, 0:512],
                                                smp_d[:, 0:512])
                        else:
                            nc.sync.dma_start(xlo[:, 0:qtr, :],
                                              xt_d[:, 0:qtr, bt0:bt0 + 512])
                            nc.gpsimd.dma_start(xlo[:, qtr:half, :],
                                                xt_d[:, qtr:half,
                                                     bt0:bt0 + 512])
                            nc.sync.dma_start(xhi[:, 0:qtr, :],
                                              xt_d[:, half:half + qtr,
                                                   bt0:bt0 + 512])
                            nc.gpsimd.dma_start(xhi[:, qtr:half, :],
                                                xt_d[:, half + qtr:cfg.NCC,
                                                     bt0:bt0 + 512])
                        xtiles[(b, tt)] = (xlo, xhi)

                    def xb_of(b, tt, ccs):
                        xlo, xhi = xtiles[(b, tt)]
                        return (xlo if ccs < half else xhi)[:, ccs % half, :]

                    def rope_pair(b, tt, Aps, Bps, d0, d1):
                        # rotA = A*C2 + B*S+-,  rotB = B*C2 + A*S-+
                        # rotA rows 0:64 -> d0[0:64]; rows 64:128 -> d1[64:]
                        # rotB rows 0:64 -> d0[64:]; rows 64:128 -> d1[0:64]
                        tl = slice(tt * 512, (tt + 1) * 512)
                        ab = rtmp.tile([128, 2, 512], bf16, name="ab",
                                       tag="ab", bufs=2)
                        nc.scalar.activation(ab[:, 0, :], Aps, Copy)
                        nc.scalar.activation(ab[:, 1, :], Bps, Copy)
                        A2, B2 = ab[:, 0, :], ab[:, 1, :]
                        m1 = rtmp.tile([128, 512], bf16, name="m1", tag="rt")
                        m2 = rtmp.tile([128, 512], bf16, name="m2", tag="rt")
                        m3 = rtmp.tile([128, 512], bf16, name="m3", tag="rt")
                        m4 = rtmp.tile([128, 512], bf16, name="m4", tag="rt")
                        nc.vector.tensor_mul(m1[:], A2, cc2_sb[:, tl])
                        nc.vector.tensor_mul(m2[:], B2, spm_sb[:, tl])
                        nc.vector.tensor_mul(m3[:], B2, cc2_sb[:, tl])
                        nc.vector.tensor_mul(m4[:], A2, smp_sb[:, tl])
                        nc.vector.tensor_add(d0[0:64, tl],
                                             m1[0:64, :], m2[0:64, :])
                        nc.vector.tensor_add(d1[64:128, tl],
                                             m1[64:128, :], m2[64:128, :])
                        rb = rtmp.tile([128, 512], bf16, name="rb", tag="rtb",
                                       bufs=2)
                        nc.vector.tensor_add(rb[:], m3[:], m4[:])
                        nc.gpsimd.dma_start(d0[64:128, tl], rb[0:64, :])
                        nc.gpsimd.dma_start(d1[0:64, tl], rb[64:128, :])

                    def v_finish(b, tt, hl, pv):
                        vf = vfm_pool.tile([128, 512], bf16,
                                           name=f"vf_{b}_{tt}_{hl}", tag="vf")
                        nc.scalar.activation(vf[:], pv, Copy)
                        nc.sync.dma_start_transpose(
                            v_sb[(b, hl)][:, tt * 4:(tt + 1) * 4, :], vf[:])

                    def b_tile_wide(b, tt):
                        """chunk-major qkv tile: 6 psum groups at once
                        (pj x2 + ps halves + po x2)."""
                        pjA = psum.tile([128, 512], f32, name=f"bqA_{b}_{tt}",
                                        tag="pj")
                        pjB = psum.tile([128, 512], f32, name=f"bqB_{b}_{tt}",
                                        tag="pj")
                        phk = psum.tile([128, 2, 512], f32, name=f"bk_{b}_{tt}",
                                        tag="ps")
                        pv = [psum.tile([128, 512], f32, name=f"bv_{b}_{tt}_{hl}",
                                        tag="po") for hl in range(2)]
                        for ccs in range(cfg.NCC):
                            xb = xb_of(b, tt, ccs)
                            st = (ccs == 0)
                            sp = (ccs == cfg.NCC - 1)
                            nc.tensor.matmul(pjA[:], wqk_sb[:, ccs, 0:128],
                                             xb, start=st, stop=sp)
                            nc.tensor.matmul(pjB[:], wqk_sb[:, ccs, 128:256],
                                             xb, start=st, stop=sp)
                            nc.tensor.matmul(phk[:, 0, :],
                                             wqk_sb[:, ccs, 256:384],
                                             xb, start=st, stop=sp)
                            nc.tensor.matmul(phk[:, 1, :],
                                             wqk_sb[:, ccs, 384:512],
                                             xb, start=st, stop=sp)
                            for hl in range(2):
                                nc.tensor.matmul(
                                    pv[hl][:],
                                    wv_sb[:, ccs, hl * 128:(hl + 1) * 128],
                                    xb, start=st, stop=sp)
                        rope_pair(b, tt, pjA[:], pjB[:],
                                  qh_sb[(b, 0)], qh_sb[(b, 1)])
                        rope_pair(b, tt, phk[:, 0, :], phk[:, 1, :],
                                  kh_sb[(b, 0)], kh_sb[(b, 1)])
                        for hl in range(2):
                            v_finish(b, tt, hl, pv[hl][:])

                    def b_tile_units(b, tt):
                        """j-major qkv tile as a list of closures, each
                        holding at most 2 psum banks (interleavable with C)."""
                        units = [lambda b=b, tt=tt: fetch_x(b, tt)]
                        state = {}

                        def jgroup(jc, into, w_sb):
                            def f():
                                p = into()
                                for ccs in range(cfg.NCC):
                                    nc.tensor.matmul(
                                        p, w_sb(ccs), xb_of(b, tt, ccs),
                                        start=(ccs == 0),
                                        stop=(ccs == cfg.NCC - 1))
                            return f

                        def qA():
                            state["qA"] = psum.tile(
                                [128, 512], f32, name=f"uqA_{b}_{tt}", tag="pj")
                            return state["qA"][:]

                        def qB():
                            state["qB"] = psum.tile(
                                [128, 512], f32, name=f"uqB_{b}_{tt}", tag="pj")
                            return state["qB"][:]

                        units.append(jgroup(0, qA,
                                     lambda c: wqk_sb[:, c, 0:128]))
                        units.append(jgroup(1, qB,
                                     lambda c: wqk_sb[:, c, 128:256]))
                        units.append(lambda: rope_pair(
                            b, tt, state["qA"][:], state["qB"][:],
                            qh_sb[(b, 0)], qh_sb[(b, 1)]))

                        def kA():
                            state["kA"] = psum.tile(
                                [128, 512], f32, name=f"ukA_{b}_{tt}", tag="pj")
                            return state["kA"][:]

                        def kB():
                            state["kB"] = psum.tile(
                                [128, 512], f32, name=f"ukB_{b}_{tt}", tag="pj")
                            return state["kB"][:]

                        units.append(jgroup(2, kA,
                                     lambda c: wqk_sb[:, c, 256:384]))
                        units.append(jgroup(3, kB,
                                     lambda c: wqk_sb[:, c, 384:512]))
                        units.append(lambda: rope_pair(
                            b, tt, state["kA"][:], state["kB"][:],
                            kh_sb[(b, 0)], kh_sb[(b, 1)]))

                        for hl in range(2):
                            def vv(hl=hl):
                                pvt = psum.tile(
                                    [128, 512], f32, name=f"uv_{b}_{tt}_{hl}",
                                    tag="pj")
                                for ccs in range(cfg.NCC):
                                    nc.tensor.matmul(
                                        pvt[:],
                                        wv_sb[:, ccs, hl * 128:(hl + 1) * 128],
                                        xb_of(b, tt, ccs),
                                        start=(ccs == 0),
                                        stop=(ccs == cfg.NCC - 1))
                                v_finish(b, tt, hl, pvt[:])
                            units.append(vv)
                        return units

                    # ---------------- attention (phase C) -----------------
                    def make_c_blocks(b):
                        blocks = []
                        for tt in range(cfg.NT):
                            n_sc = (tt + 1) * 4
                            state = {}

                            def emit_po(entry, b=b, tt=tt, n_sc=n_sc,
                                        state=state):
                                sc_, off_, pr_ = entry
                                for h in range(2):
                                    nc.tensor.matmul(
                                        state["po"][h][:, off_:],
                                        v_sb[(b, h)][:, sc_, :],
                                        pr_[:, h, off_:],
                                        start=(sc_ == 0),
                                        stop=(sc_ == n_sc - 1))

                            def block(sc, b=b, tt=tt, n_sc=n_sc,
                                      state=state, emit_po=emit_po):
                                def f():
                                    if sc == 0:
                                        state["po"] = [
                                            psum.tile([128, 512], f32,
                                                      name=f"po_{b}_{tt}_{h}",
                                                      tag="po")
                                            for h in range(2)]
                                        state["ssum"] = ssum_pool.tile(
                                            [128, 2, 512], bf16,
                                            name=f"ss_{b}_{tt}", tag="ss")
                                        state["prq"] = []
                                    off = max(0, (sc - tt * 4) * 128)
                                    sl = slice(sc * 128, (sc + 1) * 128)
                                    qsl = slice(tt * 512 + off,
                                                (tt + 1) * 512)
                                    ph = psum.tile([128, 2, 512], f32,
                                                   name=f"ps_{b}_{tt}_{sc}",
                                                   tag="ps")
                                    for h in range(2):
                                        nc.tensor.matmul(
                                            ph[:, h, off:],
                                            kh_sb[(b, h)][:, sl],
                                            qh_sb[(b, h)][:, qsl],
                                            start=True, stop=True)
                                    pr = probs_pool.tile([128, 2, 512], bf16,
                                                         name="pr", tag="pr")
                                    nc.scalar.activation(pr[:, :, off:],
                                                         ph[:, :, off:], Exp,
                                                         scale=SCALE)
                                    if sc >= tt * 4:
                                        nc.vector.tensor_mul(
                                            pr[:, :, off:off + 128],
                                            pr[:, :, off:off + 128],
                                            masks_sb[:])
                                    ssum = state["ssum"]
                                    if sc == 0:
                                        nc.vector.tensor_copy(ssum[:], pr[:])
                                    else:
                                        nc.vector.tensor_add(
                                            ssum[:, :, off:],
                                            ssum[:, :, off:],
                                            pr[:, :, off:])
                                    state["prq"].append((sc, off, pr))
                                    if len(state["prq"]) > 2:
                                        emit_po(state["prq"].pop(0))
                                return f

                            def tail(b=b, tt=tt, state=state,
                                     emit_po=emit_po):
                                while state["prq"]:
                                    emit_po(state["prq"].pop(0))
                                pd = psum.tile([128, 2, 512], f32,
                                               name=f"pd_{b}_{tt}", tag="ps")
                                ssum = state["ssum"]
                                for h in range(2):
                                    nc.tensor.matmul(pd[:, h, :], ones_sb[:],
                                                     ssum[:, h, :],
                                                     start=True, stop=True)
                                for h in range(2):
                                    rec = rec_pool.tile([128, 512], f32,
                                                        name=f"rec_{h}",
                                                        tag="rec")
                                    nc.vector.reciprocal(rec[:], pd[:, h, :])
                                    nc.vector.tensor_mul(
                                        attn_sb[(b, h)][:,
                                                        tt * 512:(tt + 1) * 512],
                                        state["po"][h][:], rec[:])

                            blocks.extend(block(sc) for sc in range(n_sc))
                            blocks.append(tail)
                        return blocks

                    # ---------------- output projection (phase D) ----------
                    # wpe is batch-independent: cache 256-wide e-slices
                    # ([128,16,256] = 8KB each, 4 live). b1 runs e-order
                    # 4,5,6,7,0,1,2,3: its first four units hit still-live
                    # tiles and the lookahead prefetch hides the reloads.
                    WPE_BUFS = 4
                    NE8 = cfg.C // 256
                    wpe_tiles = {}
                    wpe_fifo = []
                    wpe_cnt = [0]

                    d_seq = ([(0, e) for e in range(NE8)] +
                             [(1, e) for e in
                              list(range(NE8 // 2, NE8)) +
                              list(range(NE8 // 2))])

                    def issue_wpe_e(e):
                        wpe_cnt[0] += 1
                        w = persist.tile([128, cfg.GRP, 256], bf16,
                                         name=f"wpe_{rep}_{wpe_cnt[0]}",
                                         tag="wpe", bufs=WPE_BUFS)
                        for g0 in range(0, cfg.GRP, 8):
                            nc.scalar.dma_start(
                                w[:, g0:g0 + 8, :],
                                wp_d[:, g0:g0 + 8, e * 256:(e + 1) * 256])
                        wpe_tiles[e] = w
                        wpe_fifo.append(e)
                        if len(wpe_fifo) > WPE_BUFS:
                            del wpe_tiles[wpe_fifo.pop(0)]

                    def wpe_prefetch(from_idx, lookahead=3):
                        """Issue tiles for upcoming pairs; never evict a tile
                        a not-yet-emitted pair still needs."""
                        protect = {e for (_bb, e) in d_seq[from_idx:]
                                   if e in wpe_tiles}
                        for j in range(from_idx,
                                       min(from_idx + lookahead, len(d_seq))):
                            e = d_seq[j][1]
                            if e in wpe_tiles:
                                continue
                            if (len(wpe_fifo) >= WPE_BUFS
                                    and wpe_fifo[0] in protect):
                                break
                            issue_wpe_e(e)
                            protect.add(e)

                    def d_unit(b, e, hl, split=1):
                        def f():
                            idx = d_seq.index((b, e))
                            if e not in wpe_tiles:
                                issue_wpe_e(e)
                            w = wpe_tiles[e]
                            at = attn_sb[(b, hl)]
                            ew = 256 // split
                            for sp in range(split):
                                el = slice(e * 256 + sp * ew,
                                           e * 256 + (sp + 1) * ew)
                                wl = slice(sp * ew, (sp + 1) * ew)
                                pp = psum.tile([128, ew], f32,
                                               name=f"pp_{b}_{e}_{hl}_{sp}",
                                               tag="pj")
                                for u in range(cfg.GRP):
                                    nc.tensor.matmul(pp[:],
                                                     at[:, u::cfg.GRP],
                                                     w[:, u, wl],
                                                     start=(u == 0),
                                                     stop=(u == cfg.GRP - 1))
                                stg = ostg_pool.tile([128, ew], f32,
                                                     name=f"stg_{b}_{e}_{hl}",
                                                     tag="stg")
                                # DVE copy: ACT is exp-saturated in segment2,
                                # and a queued stg copy there delays the pj
                                # psum rotation (head-of-line).
                                nc.vector.tensor_copy(stg[:], pp[:])
                                oq = (nc.sync if (e + hl + sp) % 2 == 0
                                      else nc.gpsimd)
                                oq.dma_start(out_d[rep, b, hl, :, el],
                                             stg[:])
                            wpe_prefetch(idx + 1)
                        return f

                    def interleave(blocks, units, margin=4):
                        n = max(1, len(blocks) - margin)
                        per = len(units) / n
                        acc = 0.0
                        units = list(units)
                        for i, bl in enumerate(blocks):
                            bl()
                            if i < n:
                                acc += per
                            while acc >= 1.0 and units:
                                units.pop(0)()
                                acc -= 1.0
                        for u in units:
                            u()

                    # ------------------- emission order --------------------
                    fetch_x(0, 0, chunked=(rep == 0))
                    fetch_x(0, 1)
                    b_tile_wide(0, 0)
                    fetch_x(0, 2)
                    if rep == 0:
                        # emitted here so their ACT-queue issue (and hence
                        # the transfers) trail tile-0's drain copies instead
                        # of stealing DMA bandwidth from its chunk train.
                        if cfg.T > 512:
                            nc.scalar.dma_start(cc2_sb[:, 512:cfg.T],
                                                cc2_d[:, 512:cfg.T])
                            nc.scalar.dma_start(spm_sb[:, 512:cfg.T],
                                                spm_d[:, 512:cfg.T])
                            nc.scalar.dma_start(smp_sb[:, 512:cfg.T],
                                                smp_d[:, 512:cfg.T])
                        nc.scalar.dma_start(masks_sb[:], masks_d[:])
                    b_tile_wide(0, 1)
                    fetch_x(0, 3)
                    b_tile_wide(0, 2)
                    b_tile_wide(0, 3)

                    wpe_prefetch(0, lookahead=2)
                    b1_units = []
                    for tt in range(cfg.NT):
                        b1_units.extend(b_tile_units(1, tt))
                    interleave(make_c_blocks(0), b1_units, margin=6)

                    d0_units = [d_unit(0, e, hl)
                                for e in range(NE8) for hl in range(2)]
                    # margin=1: d0 fillers (pj tag only — no po conflict)
                    # run right through C(b1)'s tail, covering the
                    # attn-mul drain that gates D(b1)'s first LDW.
                    interleave(make_c_blocks(1), d0_units, margin=-9)

                    for _b1, e in d_seq[NE8:]:
                        for hl in range(2):
                            d_unit(1, e, hl)()

    nc.compile()
    return nc


# =====================================================================
# Host-side input prep / output gather
# =====================================================================

def _part_major(a2d, ncc):
    """[ncc*128, F] -> [128, ncc, F] with row r = chunk*128 + p."""
    F = a2d.shape[1]
    return np.ascontiguousarray(
        a2d.reshape(ncc, 128, F).transpose(1, 0, 2))


def make_trig(cfg: Cfg):
    pos = np.arange(cfg.T, dtype=np.float64)[None, :]        # [1,T]
    j = np.arange(64, dtype=np.float64)[:, None]             # [64,1]
    inv = ROPE_BASE ** (-2.0 * j / Dh)
    ang = pos * inv                                          # [64,T]
    sin = np.sin(ang).astype(np.float32)
    cos = np.cos(ang).astype(np.float32)
    cc2 = np.concatenate([cos, cos], axis=0).astype(BF16)    # [128,T]
    spm = np.concatenate([-sin, sin], axis=0).astype(BF16)
    smp = np.concatenate([sin, -sin], axis=0).astype(BF16)
    return cc2, spm, smp


def make_masks():
    # one lower-triangular 128x128 block mask, duplicated for the 2 heads:
    # within a diagonal block the valid region is local col j >= partition p.
    p = np.arange(128)[:, None]
    jj = np.arange(128)[None, :]
    tri = (p <= jj)
    return np.stack([tri, tri], axis=1).astype(BF16)         # [128,2,128]


def make_in_maps(x, w_qkv, w_proj, cfg: Cfg = FULL, n_cores=N_CORES,
                 n_head=N_HEAD):
    x = np.asarray(x, np.float32)
    w_qkv = np.asarray(w_qkv, np.float32)
    w_proj = np.asarray(w_proj, np.float32)
    Cm = cfg.C

    xT = np.ascontiguousarray(x.reshape(cfg.BT, Cm).T)       # [C, BT]
    xt = _part_major(xT, cfg.NCC).astype(BF16)
    wp = _part_major(w_proj, cfg.GRP).astype(BF16)
    cc2, spm, smp = make_trig(cfg)
    masks = make_masks()

    wq = w_qkv[:, 0:Cm]
    wk = w_qkv[:, Cm:2 * Cm]
    wv_all = w_qkv[:, 2 * Cm:3 * Cm]

    in_maps = []
    for c in range(n_cores):
        h0, h1 = 2 * c, 2 * c + 1
        q0 = wq[:, h0 * 128:(h0 + 1) * 128]
        q1 = wq[:, h1 * 128:(h1 + 1) * 128]
        k0 = wk[:, h0 * 128:(h0 + 1) * 128]
        k1 = wk[:, h1 * 128:(h1 + 1) * 128]
        qA = np.concatenate([q0[:, 0:64], q1[:, 64:128]], axis=1)
        qB = np.concatenate([q0[:, 64:128], q1[:, 0:64]], axis=1)
        kA = np.concatenate([k0[:, 0:64], k1[:, 64:128]], axis=1)
        kB = np.concatenate([k0[:, 64:128], k1[:, 0:64]], axis=1)
        wqk = _part_major(
            np.concatenate([qA, qB, kA, kB], axis=1), cfg.NCC).astype(BF16)
        wv = _part_major(
            np.concatenate([wv_all[:, h0 * 128:(h0 + 1) * 128],
                            wv_all[:, h1 * 128:(h1 + 1) * 128]], axis=1),
            cfg.NCC).astype(BF16)
        in_maps.append(dict(xt=xt, wqk=wqk, wv=wv, wp=wp,
                            cc2=cc2, spm=spm, smp=smp, masks=masks))
    return in_maps


def gather(outs, cfg: Cfg = FULL):
    """outs: per-core [rep, B, H_LOCAL, 128, C] -> full [B, T, C]."""
    rows = np.concatenate(
        [o[-1].reshape(cfg.B, H_LOCAL * 128, cfg.C) for o in outs], axis=1)
    return np.ascontiguousarray(rows.reshape(cfg.B, cfg.T, cfg.C))


# =====================================================================
# Public entry point
# =====================================================================

_NC_CACHE = {}


def get_nc(debug=False):
    key = ("full", debug)
    if key not in _NC_CACHE:
        _NC_CACHE[key] = build_nc(FULL, debug=debug)
    return _NC_CACHE[key]


def kernel(x, w_qkv, w_proj):
    from concourse.bass_utils import run_bass_kernel_spmd
    nc = get_nc()
    in_maps = make_in_maps(x, w_qkv, w_proj)
    res = run_bass_kernel_spmd(nc, in_maps, list(range(N_CORES)))
    return gather([res.results[c]["out"] for c in range(N_CORES)])



# revision 28
# speedup vs baseline: 1.5289x; 1.5289x over previous
"""Self-contained Trainium2 Bass kernel for nn_MultiHeadAttention_71528385347884.

Strategy: head tensor-parallel across 8 cores (2 heads/core). Per core:
  - QKV projection with x transposed (feature-major q/k, token-major v)
  - RoPE via host-side A/B weight-column packing (no cross-partition ops)
  - causal attention in [s,t] score layout, softmax without max-subtraction
    (scores are bounded ~|4.5|), denominator via all-ones matmul
  - output projection exploits the reference's scrambled
    transpose(0,2,1,3).reshape(B,T,C): each core produces disjoint output
    rows -> host gather is pure concatenation.
"""

import math
import numpy as np
import ml_dtypes

# ---- problem constants (hardcoded; kernel.py must not read spec/reference) ----
B = 2
T = 2048          # sequence length per batch
C = 2048          # model dim
Dh = 128          # head dim
N_HEAD = 16
N_CORES = 8
H_LOCAL = 2       # heads per core
ROPE_BASE = 10000.0
SCALE = 1.0 / math.sqrt(Dh)

BF16 = ml_dtypes.bfloat16


class Cfg:
    """Size parameters so the same builder runs a small CoreSim config."""

    def __init__(self, B=B, T=T, C=C):
        assert T % 512 == 0 and C % 128 == 0
        self.B = B
        self.T = T
        self.C = C
        self.NCC = C // 128        # contraction chunks for qkv matmuls
        self.BT = B * T
        self.NT = T // 512         # 512-wide t-tiles per batch
        self.GRP = C // Dh         # tokens folded per output row by the reshape
        self.TAU = T // self.GRP   # output rows per (b, h); must be 128
        assert self.TAU == 128
        self.ET = max(1, C // 512)  # 512-wide e-tiles of the output
        self.JQK = 4 * 128         # qA,qB,kA,kB feature blocks
        self.JV = H_LOCAL * 128


FULL = Cfg()


# =====================================================================
# Device program builder
# =====================================================================

def build_nc(cfg: Cfg, debug=False, repeat=1, phases="BCD", loop_repeat=None):
    import contextlib
    import concourse.bass as bass
    import concourse.mybir as mybir
    import concourse.tile as tile
    from concourse import bacc

    f32 = mybir.dt.float32
    bf16 = mybir.dt.bfloat16
    Exp = mybir.ActivationFunctionType.Exp
    Copy = mybir.ActivationFunctionType.Copy

    nc = bacc.Bacc(None, target_bir_lowering=False, debug=debug)

    xt_d = nc.dram_tensor("xt", [128, cfg.NCC, cfg.BT], bf16, kind="ExternalInput")
    wqk_d = nc.dram_tensor("wqk", [128, cfg.NCC, cfg.JQK], bf16, kind="ExternalInput")
    wv_d = nc.dram_tensor("wv", [128, cfg.NCC, cfg.JV], bf16, kind="ExternalInput")
    wp_d = nc.dram_tensor("wp", [128, cfg.GRP, cfg.C], bf16, kind="ExternalInput")
    cc2_d = nc.dram_tensor("cc2", [128, cfg.T], bf16, kind="ExternalInput")
    spm_d = nc.dram_tensor("spm", [128, cfg.T], bf16, kind="ExternalInput")
    smp_d = nc.dram_tensor("smp", [128, cfg.T], bf16, kind="ExternalInput")
    masks_d = nc.dram_tensor("masks", [128, 2, 128], bf16,
                             kind="ExternalInput")
    # repeat>1 (bench-only) gets a per-rep output slice so no rep's stores
    # are dead — guards the marginal-rep timing against compiler DCE.
    out_d = nc.dram_tensor("out", [repeat, cfg.B, H_LOCAL, 128, cfg.C], f32,
                           kind="ExternalOutput")

    with tile.TileContext(nc) as tc:
        with tc.tile_pool(name="persist", bufs=1) as persist:
            # ---- persistent SBUF state ----
            wqk_sb = persist.tile([128, cfg.NCC, cfg.JQK], bf16, name="wqk_sb",
                                  tag="wqk_sb")
            wv_sb = persist.tile([128, cfg.NCC, cfg.JV], bf16, name="wv_sb",
                                 tag="wv_sb")
            cc2_sb = persist.tile([128, cfg.T], bf16, name="cc2_sb", tag="cc2_sb")
            spm_sb = persist.tile([128, cfg.T], bf16, name="spm_sb", tag="spm_sb")
            smp_sb = persist.tile([128, cfg.T], bf16, name="smp_sb", tag="smp_sb")
            masks_sb = persist.tile([128, 2, 128], bf16, name="masks_sb",
                                    tag="masks_sb")
            ones_sb = persist.tile([128, 128], bf16, name="ones_sb", tag="ones_sb")

            nc.vector.memset(ones_sb[:], 1.0)

            # per-(b, head-or-tile) persistent tensors; q/k are stored
            # head-contiguous ([dims 0:128 of head h] on partitions) so the
            # score matmuls contract K=128 in one shot.
            qh_sb, kh_sb = {}, {}
            v_sb, attn_sb = {}, {}
            for b in range(cfg.B):
                for hl in range(H_LOCAL):
                    qh_sb[(b, hl)] = persist.tile([128, cfg.T], bf16,
                                                  name=f"qh_{b}_{hl}",
                                                  tag=f"qh_{b}_{hl}")
                    kh_sb[(b, hl)] = persist.tile([128, cfg.T], bf16,
                                                  name=f"kh_{b}_{hl}",
                                                  tag=f"kh_{b}_{hl}")
                for hl in range(H_LOCAL):
                    v_sb[(b, hl)] = persist.tile(
                        [128, cfg.T // 128, 128], bf16,
                        name=f"v_{b}_{hl}", tag=f"v_{b}_{hl}")
                    attn_sb[(b, hl)] = persist.tile(
                        [128, cfg.T], bf16,
                        name=f"at_{b}_{hl}", tag=f"at_{b}_{hl}")


            _loop_es = contextlib.ExitStack()
            if loop_repeat:
                # steady-state timing mode: weights/trig loaded in a
                # prologue, then For_i repeats one rep body on-device.
                nc.scalar.dma_start(wqk_sb[:], wqk_d[:])
                nc.scalar.dma_start(wv_sb[:], wv_d[:])
                nc.scalar.dma_start(cc2_sb[:], cc2_d[:])
                nc.scalar.dma_start(spm_sb[:], spm_d[:])
                nc.scalar.dma_start(smp_sb[:], smp_d[:])
                nc.scalar.dma_start(masks_sb[:], masks_d[:])
                if "B" not in phases:
                    for b in range(cfg.B):
                        for hl in range(H_LOCAL):
                            nc.vector.memset(qh_sb[(b, hl)][:], 0.01)
                            nc.vector.memset(kh_sb[(b, hl)][:], 0.01)
                            nc.vector.memset(v_sb[(b, hl)][:], 0.01)
                if "D" in phases and "C" not in phases:
                    for b in range(cfg.B):
                        for hl in range(H_LOCAL):
                            nc.vector.memset(attn_sb[(b, hl)][:], 0.01)
                _loop_es.enter_context(tc.For_i(0, loop_repeat))

            for rep in range(repeat):
                # Phase plan (PE keeps busy through attention's exp waits):
                #   B(b0) -> C(b0) interleaved with B(b1) -> C(b1)
                #   interleaved with D(b0) -> D(b1).
                # One unified PSUM pool, 8 banks exactly:
                #   pj [128,512]x2 (B qkv groups, D proj groups)
                #   ps [128,2,512]x2 (C scores + ones-denominator)
                #   po [128,512]x2 (C attn accumulators)
                with (
                    tc.tile_pool(name=f"xb_pool{rep}", bufs=4) as xb_pool,
                    tc.tile_pool(name=f"rtmp{rep}", bufs=4) as rtmp,
                    tc.tile_pool(name=f"probs{rep}", bufs=5) as probs_pool,
                    tc.tile_pool(name=f"ssum{rep}", bufs=2) as ssum_pool,
                    tc.tile_pool(name=f"rec{rep}", bufs=2) as rec_pool,
                    tc.tile_pool(name=f"ostg{rep}", bufs=3) as ostg_pool,
                    tc.tile_pool(name=f"vfm{rep}", bufs=4) as vfm_pool,
                    tc.tile_pool(name=f"ps{rep}", bufs=2, space="PSUM") as psum,
                ):
                    half = cfg.NCC // 2
                    qtr = max(1, half // 2)
                    xtiles = {}

                    def fetch_x(b, tt, chunked=False):
                        bt0 = b * cfg.T + tt * 512
                        xlo = xb_pool.tile([128, half, 512], bf16,
                                           name=f"xbl_{b}_{tt}", tag="xb")
                        xhi = xb_pool.tile([128, half, 512], bf16,
                                           name=f"xbh_{b}_{tt}", tag="xb")
                        if chunked:
                            # startup: single-chunk DMAs for the first 4
                            # (fast first matmul), 2-chunk after (halve the
                            # ~0.6us-per-op issue load on the queues).
                            steps = [(0, 1), (1, 1), (2, 2),
                                     (4, 4), (8, 8)]
                            for c, w in steps:
                                xdst = (xlo if c < half else xhi)
                                par = (c // w) % 2 == 0
                                xq = (nc.sync if par else nc.gpsimd)
                                wq = (nc.gpsimd if par else nc.sync)
                                nc.scalar.dma_start(wqk_sb[:, c:c + w, :],
                                                    wqk_d[:, c:c + w, :])
                                wq.dma_start(wv_sb[:, c:c + w, :],
                                             wv_d[:, c:c + w, :])
                                cl = c % half
                                xq.dma_start(
                                    xdst[:, cl:cl + w, :],
                                    xt_d[:, c:c + w, bt0:bt0 + 512])
                            nc.scalar.dma_start(cc2_sb[:, 0:512],
                                                cc2_d[:, 0:512])
                            nc.scalar.dma_start(spm_sb[:, 0:512],
                                                spm_d[:, 0:512])
                            nc.scalar.dma_start(smp_sb[:, 0:512],
                                                smp_d[:, 0:512])
                        else:
                            nc.sync.dma_start(xlo[:, 0:qtr, :],
                                              xt_d[:, 0:qtr, bt0:bt0 + 512])
                            nc.gpsimd.dma_start(xlo[:, qtr:half, :],
                                                xt_d[:, qtr:half,
                                                     bt0:bt0 + 512])
                            nc.sync.dma_start(xhi[:, 0:qtr, :],
                                              xt_d[:, half:half + qtr,
                                                   bt0:bt0 + 512])
                            nc.gpsimd.dma_start(xhi[:, qtr:half, :],
                                                xt_d[:, half + qtr:cfg.NCC,
                                                     bt0:bt0 + 512])
                        xtiles[(b, tt)] = (xlo, xhi)

                    def xb_of(b, tt, ccs):
                        xlo, xhi = xtiles[(b, tt)]
                        return (xlo if ccs < half else xhi)[:, ccs % half, :]

                    def rope_pair(b, tt, Aps, Bps, d0, d1):
                        # rotA = A*C2 + B*S+-,  rotB = B*C2 + A*S-+
                        # rotA rows 0:64 -> d0[0:64]; rows 64:128 -> d1[64:]
                        # rotB rows 0:64 -> d0[64:]; rows 64:128 -> d1[0:64]
                        # flat 2D APs throughout: 3D / partial-partition APs
                        # run 3-8x slower on ACT/DVE (HW slow path).
                        tl = slice(tt * 512, (tt + 1) * 512)
                        a2 = rtmp.tile([128, 512], bf16, name="a2",
                                       tag="ab", bufs=4)
                        b2 = rtmp.tile([128, 512], bf16, name="b2",
                                       tag="ab", bufs=4)
                        nc.scalar.activation(a2[:], Aps, Copy)
                        nc.scalar.activation(b2[:], Bps, Copy)
                        m1 = rtmp.tile([128, 512], bf16, name="m1", tag="rt")
                        m2 = rtmp.tile([128, 512], bf16, name="m2", tag="rt")
                        m3 = rtmp.tile([128, 512], bf16, name="m3", tag="rt")
                        m4 = rtmp.tile([128, 512], bf16, name="m4", tag="rt")
                        nc.vector.tensor_mul(m1[:], a2[:], cc2_sb[:, tl])
                        nc.vector.tensor_mul(m2[:], b2[:], spm_sb[:, tl])
                        nc.vector.tensor_mul(m3[:], b2[:], cc2_sb[:, tl])
                        nc.vector.tensor_mul(m4[:], a2[:], smp_sb[:, tl])
                        nc.vector.tensor_add(d0[0:64, tl],
                                             m1[0:64, :], m2[0:64, :])
                        nc.vector.tensor_add(d1[64:128, tl],
                                             m1[64:128, :], m2[64:128, :])
                        rb = rtmp.tile([128, 512], bf16, name="rb", tag="rtb",
                                       bufs=2)
                        nc.vector.tensor_add(rb[:], m3[:], m4[:])
                        nc.gpsimd.dma_start(d0[64:128, tl], rb[0:64, :])
                        nc.gpsimd.dma_start(d1[0:64, tl], rb[64:128, :])

                    def v_finish(b, tt, hl, pv):
                        vf = vfm_pool.tile([128, 512], bf16,
                                           name=f"vf_{b}_{tt}_{hl}", tag="vf")
                        nc.scalar.activation(vf[:], pv, Copy)
                        nc.sync.dma_start_transpose(
                            v_sb[(b, hl)][:, tt * 4:(tt + 1) * 4, :], vf[:])

                    def b_tile_wide(b, tt):
                        """chunk-major qkv tile: 6 psum groups at once
                        (pj x2 + ps halves + po x2)."""
                        pjA = psum.tile([128, 512], f32, name=f"bqA_{b}_{tt}",
                                        tag="pj")
                        pjB = psum.tile([128, 512], f32, name=f"bqB_{b}_{tt}",
                                        tag="pj")
                        phk = psum.tile([128, 2, 512], f32,
                                        name=f"bk_{b}_{tt}", tag="ps")
                        pv = [psum.tile([128, 512], f32, name=f"bv_{b}_{tt}_{hl}",
                                        tag="po") for hl in range(2)]
                        for ccs in range(cfg.NCC):
                            xb = xb_of(b, tt, ccs)
                            st = (ccs == 0)
                            sp = (ccs == cfg.NCC - 1)
                            nc.tensor.matmul(pjA[:], wqk_sb[:, ccs, 0:128],
                                             xb, start=st, stop=sp)
                            nc.tensor.matmul(pjB[:], wqk_sb[:, ccs, 128:256],
                                             xb, start=st, stop=sp)
                            nc.tensor.matmul(phk[:, 0, :],
                                             wqk_sb[:, ccs, 256:384],
                                             xb, start=st, stop=sp)
                            nc.tensor.matmul(phk[:, 1, :],
                                             wqk_sb[:, ccs, 384:512],
                                             xb, start=st, stop=sp)
                            for hl in range(2):
                                nc.tensor.matmul(
                                    pv[hl][:],
                                    wv_sb[:, ccs, hl * 128:(hl + 1) * 128],
                                    xb, start=st, stop=sp)
                        rope_pair(b, tt, pjA[:], pjB[:],
                                  qh_sb[(b, 0)], qh_sb[(b, 1)])
                        rope_pair(b, tt, phk[:, 0, :], phk[:, 1, :],
                                  kh_sb[(b, 0)], kh_sb[(b, 1)])
                        for hl in range(2):
                            v_finish(b, tt, hl, pv[hl][:])

                    def b_tile_units(b, tt):
                        """j-major qkv tile as a list of closures, each
                        holding at most 2 psum banks (interleavable with C)."""
                        units = [lambda b=b, tt=tt: fetch_x(b, tt)]
                        state = {}

                        def jgroup(jc, into, w_sb):
                            def f():
                                p = into()
                                for ccs in range(cfg.NCC):
                                    nc.tensor.matmul(
                                        p, w_sb(ccs), xb_of(b, tt, ccs),
                                        start=(ccs == 0),
                                        stop=(ccs == cfg.NCC - 1))
                            return f

                        def qA():
                            state["qA"] = psum.tile(
                                [128, 512], f32, name=f"uqA_{b}_{tt}", tag="pj")
                            return state["qA"][:]

                        def qB():
                            state["qB"] = psum.tile(
                                [128, 512], f32, name=f"uqB_{b}_{tt}", tag="pj")
                            return state["qB"][:]

                        units.append(jgroup(0, qA,
                                     lambda c: wqk_sb[:, c, 0:128]))
                        units.append(jgroup(1, qB,
                                     lambda c: wqk_sb[:, c, 128:256]))
                        units.append(lambda: rope_pair(
                            b, tt, state["qA"][:], state["qB"][:],
                            qh_sb[(b, 0)], qh_sb[(b, 1)]))

                        def kA():
                            state["kA"] = psum.tile(
                                [128, 512], f32, name=f"ukA_{b}_{tt}", tag="pj")
                            return state["kA"][:]

                        def kB():
                            state["kB"] = psum.tile(
                                [128, 512], f32, name=f"ukB_{b}_{tt}", tag="pj")
                            return state["kB"][:]

                        units.append(jgroup(2, kA,
                                     lambda c: wqk_sb[:, c, 256:384]))
                        units.append(jgroup(3, kB,
                                     lambda c: wqk_sb[:, c, 384:512]))
                        units.append(lambda: rope_pair(
                            b, tt, state["kA"][:], state["kB"][:],
                            kh_sb[(b, 0)], kh_sb[(b, 1)]))

                        for hl in range(2):
                            def vv(hl=hl):
                                pvt = psum.tile(
                                    [128, 512], f32, name=f"uv_{b}_{tt}_{hl}",
                                    tag="pj")
                                for ccs in range(cfg.NCC):
                                    nc.tensor.matmul(
                                        pvt[:],
                                        wv_sb[:, ccs, hl * 128:(hl + 1) * 128],
                                        xb_of(b, tt, ccs),
                                        start=(ccs == 0),
                                        stop=(ccs == cfg.NCC - 1))
                                v_finish(b, tt, hl, pvt[:])
                            units.append(vv)
                        return units

                    # ---------------- attention (phase C) -----------------
                    def make_c_blocks(b):
                        blocks = []
                        for tt in range(cfg.NT):
                            n_sc = (tt + 1) * 4
                            state = {}

                            def emit_po(entry, b=b, tt=tt, n_sc=n_sc,
                                        state=state):
                                sc_, off_, pr_ = entry
                                for h in range(2):
                                    nc.tensor.matmul(
                                        state["po"][h][:, off_:],
                                        v_sb[(b, h)][:, sc_, :],
                                        pr_[:, h, off_:],
                                        start=(sc_ == 0),
                                        stop=(sc_ == n_sc - 1))

                            def block(sc, b=b, tt=tt, n_sc=n_sc,
                                      state=state, emit_po=emit_po):
                                def f():
                                    if sc == 0:
                                        state["po"] = [
                                            psum.tile([128, 512], f32,
                                                      name=f"po_{b}_{tt}_{h}",
                                                      tag="po")
                                            for h in range(2)]
                                        state["ssum"] = ssum_pool.tile(
                                            [128, 2, 512], bf16,
                                            name=f"ss_{b}_{tt}", tag="ss")
                                        state["prq"] = []
                                    off = max(0, (sc - tt * 4) * 128)
                                    sl = slice(sc * 128, (sc + 1) * 128)
                                    qsl = slice(tt * 512 + off,
                                                (tt + 1) * 512)
                                    ph = psum.tile([128, 2, 512], f32,
                                                   name=f"ps_{b}_{tt}_{sc}",
                                                   tag="ps")
                                    for h in range(2):
                                        nc.tensor.matmul(
                                            ph[:, h, off:],
                                            kh_sb[(b, h)][:, sl],
                                            qh_sb[(b, h)][:, qsl],
                                            start=True, stop=True)
                                    pr = probs_pool.tile([128, 2, 512], bf16,
                                                         name="pr", tag="pr")
                                    nc.scalar.activation(pr[:, :, off:],
                                                         ph[:, :, off:], Exp,
                                                         scale=SCALE)
                                    if sc >= tt * 4:
                                        nc.vector.tensor_mul(
                                            pr[:, :, off:off + 128],
                                            pr[:, :, off:off + 128],
                                            masks_sb[:])
                                    ssum = state["ssum"]
                                    if sc == 0:
                                        nc.vector.tensor_copy(ssum[:], pr[:])
                                    else:
                                        nc.vector.tensor_add(
                                            ssum[:, :, off:],
                                            ssum[:, :, off:],
                                            pr[:, :, off:])
                                    state["prq"].append((sc, off, pr))
                                    if len(state["prq"]) > 2:
                                        emit_po(state["prq"].pop(0))
                                return f

                            def tail(b=b, tt=tt, state=state,
                                     emit_po=emit_po):
                                while state["prq"]:
                                    emit_po(state["prq"].pop(0))
                                pd = psum.tile([128, 2, 512], f32,
                                               name=f"pd_{b}_{tt}", tag="ps")
                                ssum = state["ssum"]
                                for h in range(2):
                                    nc.tensor.matmul(pd[:, h, :], ones_sb[:],
                                                     ssum[:, h, :],
                                                     start=True, stop=True)
                                for h in range(2):
                                    rec = rec_pool.tile([128, 512], f32,
                                                        name=f"rec_{h}",
                                                        tag="rec")
                                    nc.vector.reciprocal(rec[:], pd[:, h, :])
                                    nc.vector.tensor_mul(
                                        attn_sb[(b, h)][:,
                                                        tt * 512:(tt + 1) * 512],
                                        state["po"][h][:], rec[:])

                            blocks.extend(block(sc) for sc in range(n_sc))
                            blocks.append(tail)
                        return blocks

                    # ---------------- output projection (phase D) ----------
                    # wpe is batch-independent: cache 256-wide e-slices
                    # ([128,16,256] = 8KB each, 4 live). b1 runs e-order
                    # 4,5,6,7,0,1,2,3: its first four units hit still-live
                    # tiles and the lookahead prefetch hides the reloads.
                    WPE_BUFS = 4
                    NE8 = cfg.C // 256
                    wpe_tiles = {}
                    wpe_fifo = []
                    wpe_cnt = [0]

                    d_seq = ([(0, e) for e in range(NE8)] +
                             [(1, e) for e in
                              list(range(NE8 // 2, NE8)) +
                              list(range(NE8 // 2))])

                    def issue_wpe_e(e):
                        wpe_cnt[0] += 1
                        w = persist.tile([128, cfg.GRP, 256], bf16,
                                         name=f"wpe_{rep}_{wpe_cnt[0]}",
                                         tag="wpe", bufs=WPE_BUFS)
                        for g0 in range(0, cfg.GRP, 8):
                            nc.scalar.dma_start(
                                w[:, g0:g0 + 8, :],
                                wp_d[:, g0:g0 + 8, e * 256:(e + 1) * 256])
                        wpe_tiles[e] = w
                        wpe_fifo.append(e)
                        if len(wpe_fifo) > WPE_BUFS:
                            del wpe_tiles[wpe_fifo.pop(0)]

                    def wpe_prefetch(from_idx, lookahead=3):
                        """Issue tiles for upcoming pairs; never evict a tile
                        a not-yet-emitted pair still needs."""
                        protect = {e for (_bb, e) in d_seq[from_idx:]
                                   if e in wpe_tiles}
                        for j in range(from_idx,
                                       min(from_idx + lookahead, len(d_seq))):
                            e = d_seq[j][1]
                            if e in wpe_tiles:
                                continue
                            if (len(wpe_fifo) >= WPE_BUFS
                                    and wpe_fifo[0] in protect):
                                break
                            issue_wpe_e(e)
                            protect.add(e)

                    def d_unit(b, e, hl, split=1):
                        def f():
                            idx = d_seq.index((b, e))
                            if e not in wpe_tiles:
                                issue_wpe_e(e)
                            w = wpe_tiles[e]
                            at = attn_sb[(b, hl)]
                            ew = 256 // split
                            for sp in range(split):
                                el = slice(e * 256 + sp * ew,
                                           e * 256 + (sp + 1) * ew)
                                wl = slice(sp * ew, (sp + 1) * ew)
                                pp = psum.tile([128, ew], f32,
                                               name=f"pp_{b}_{e}_{hl}_{sp}",
                                               tag="pj")
                                for u in range(cfg.GRP):
                                    nc.tensor.matmul(pp[:],
                                                     at[:, u::cfg.GRP],
                                                     w[:, u, wl],
                                                     start=(u == 0),
                                                     stop=(u == cfg.GRP - 1))
                                stg = ostg_pool.tile([128, ew], f32,
                                                     name=f"stg_{b}_{e}_{hl}",
                                                     tag="stg")
                                # DVE copy: ACT is exp-saturated in segment2,
                                # and a queued stg copy there delays the pj
                                # psum rotation (head-of-line).
                                nc.vector.tensor_copy(stg[:], pp[:])
                                oq = (nc.sync if (e + hl + sp) % 2 == 0
                                      else nc.gpsimd)
                                oq.dma_start(out_d[rep, b, hl, :, el],
                                             stg[:])
                            wpe_prefetch(idx + 1)
                        return f

                    def interleave(blocks, units, margin=4):
                        n = max(1, len(blocks) - margin)
                        per = len(units) / n
                        acc = 0.0
                        units = list(units)
                        for i, bl in enumerate(blocks):
                            bl()
                            if i < n:
                                acc += per
                            while acc >= 1.0 and units:
                                units.pop(0)()
                                acc -= 1.0
                        for u in units:
                            u()

                    # ------------------- emission order --------------------
                    first = (rep == 0) and loop_repeat is None

                    def dummy_out(src_tile):
                        stg = ostg_pool.tile([128, 512], f32,
                                             name="dstg", tag="stg")
                        nc.vector.tensor_copy(stg[:], src_tile)
                        nc.sync.dma_start(out_d[rep, 0, 0, :, 0:512], stg[:])

                    if "B" in phases:
                        fetch_x(0, 0, chunked=first)
                        fetch_x(0, 1)
                        b_tile_wide(0, 0)
                        fetch_x(0, 2)
                        if first:
                            # emitted here so their ACT-queue issue (and hence
                            # the transfers) trail tile-0's drain copies
                            # instead of stealing DMA bandwidth from its
                            # chunk train.
                            if cfg.T > 512:
                                nc.scalar.dma_start(cc2_sb[:, 512:cfg.T],
                                                    cc2_d[:, 512:cfg.T])
                                nc.scalar.dma_start(spm_sb[:, 512:cfg.T],
                                                    spm_d[:, 512:cfg.T])
                                nc.scalar.dma_start(smp_sb[:, 512:cfg.T],
                                                    smp_d[:, 512:cfg.T])
                            nc.scalar.dma_start(masks_sb[:], masks_d[:])
                        b_tile_wide(0, 1)
                        fetch_x(0, 3)
                        b_tile_wide(0, 2)
                        b_tile_wide(0, 3)
                    elif first:
                        nc.scalar.dma_start(masks_sb[:], masks_d[:])

                    if phases == "BCD":
                        wpe_prefetch(0, lookahead=2)
                        b1_units = []
                        for tt in range(cfg.NT):
                            b1_units.extend(b_tile_units(1, tt))
                        interleave(make_c_blocks(0), b1_units, margin=6)

                        d0_units = [d_unit(0, e, hl)
                                    for e in range(NE8) for hl in range(2)]
                        # margin=1: d0 fillers (pj tag only — no po conflict)
                        # run right through C(b1)'s tail, covering the
                        # attn-mul drain that gates D(b1)'s first LDW.
                        interleave(make_c_blocks(1), d0_units, margin=-9)

                        for _b1, e in d_seq[NE8:]:
                            for hl in range(2):
                                d_unit(1, e, hl)()
                    else:
                        if "B" in phases:
                            b1_units = []
                            for tt in range(cfg.NT):
                                b1_units.extend(b_tile_units(1, tt))
                            if "C" in phases:
                                interleave(make_c_blocks(0), b1_units,
                                           margin=6)
                            else:
                                for u in b1_units:
                                    u()
                        elif "C" in phases:
                            for bl in make_c_blocks(0):
                                bl()
                        if "C" in phases:
                            for bl in make_c_blocks(1):
                                bl()
                        if "D" in phases:
                            wpe_prefetch(0, lookahead=2)
                            for b in range(cfg.B):
                                for e in range(NE8):
                                    for hl in range(2):
                                        d_unit(b, e, hl)()
                        elif "C" in phases:
                            dummy_out(attn_sb[(1, 1)][:, 0:512])
                        else:
                            dummy_out(qh_sb[(1, 0)][:, 0:512])

            _loop_es.close()

    nc.compile()
    return nc


# =====================================================================
# Host-side input prep / output gather
# =====================================================================

def _part_major(a2d, ncc):
    """[ncc*128, F] -> [128, ncc, F] with row r = chunk*128 + p."""
    F = a2d.shape[1]
    return np.ascontiguousarray(
        a2d.reshape(ncc, 128, F).transpose(1, 0, 2))


def make_trig(cfg: Cfg):
    pos = np.arange(cfg.T, dtype=np.float64)[None, :]        # [1,T]
    j = np.arange(64, dtype=np.float64)[:, None]             # [64,1]
    inv = ROPE_BASE ** (-2.0 * j / Dh)
    ang = pos * inv                                          # [64,T]
    sin = np.sin(ang).astype(np.float32)
    cos = np.cos(ang).astype(np.float32)
    cc2 = np.concatenate([cos, cos], axis=0).astype(BF16)    # [128,T]
    spm = np.concatenate([-sin, sin], axis=0).astype(BF16)
    smp = np.concatenate([sin, -sin], axis=0).astype(BF16)
    return cc2, spm, smp


def make_masks():
    # one lower-triangular 128x128 block mask, duplicated for the 2 heads:
    # within a diagonal block the valid region is local col j >= partition p.
    p = np.arange(128)[:, None]
    jj = np.arange(128)[None, :]
    tri = (p <= jj)
    return np.stack([tri, tri], axis=1).astype(BF16)         # [128,2,128]


def make_in_maps(x, w_qkv, w_proj, cfg: Cfg = FULL, n_cores=N_CORES,
                 n_head=N_HEAD):
    x = np.asarray(x, np.float32)
    w_qkv = np.asarray(w_qkv, np.float32)
    w_proj = np.asarray(w_proj, np.float32)
    Cm = cfg.C

    xT = np.ascontiguousarray(x.reshape(cfg.BT, Cm).T)       # [C, BT]
    xt = _part_major(xT, cfg.NCC).astype(BF16)
    wp = _part_major(w_proj, cfg.GRP).astype(BF16)
    cc2, spm, smp = make_trig(cfg)
    masks = make_masks()

    wq = w_qkv[:, 0:Cm]
    wk = w_qkv[:, Cm:2 * Cm]
    wv_all = w_qkv[:, 2 * Cm:3 * Cm]

    in_maps = []
    for c in range(n_cores):
        h0, h1 = 2 * c, 2 * c + 1
        q0 = wq[:, h0 * 128:(h0 + 1) * 128]
        q1 = wq[:, h1 * 128:(h1 + 1) * 128]
        k0 = wk[:, h0 * 128:(h0 + 1) * 128]
        k1 = wk[:, h1 * 128:(h1 + 1) * 128]
        qA = np.concatenate([q0[:, 0:64], q1[:, 64:128]], axis=1)
        qB = np.concatenate([q0[:, 64:128], q1[:, 0:64]], axis=1)
        kA = np.concatenate([k0[:, 0:64], k1[:, 64:128]], axis=1)
        kB = np.concatenate([k0[:, 64:128], k1[:, 0:64]], axis=1)
        wqk = _part_major(
            np.concatenate([qA, qB, kA, kB], axis=1), cfg.NCC).astype(BF16)
        wv = _part_major(
            np.concatenate([wv_all[:, h0 * 128:(h0 + 1) * 128],
                            wv_all[:, h1 * 128:(h1 + 1) * 128]], axis=1),
            cfg.NCC).astype(BF16)
        in_maps.append(dict(xt=xt, wqk=wqk, wv=wv, wp=wp,
                            cc2=cc2, spm=spm, smp=smp, masks=masks))
    return in_maps


def gather(outs, cfg: Cfg = FULL):
    """outs: per-core [rep, B, H_LOCAL, 128, C] -> full [B, T, C]."""
    rows = np.concatenate(
        [o[-1].reshape(cfg.B, H_LOCAL * 128, cfg.C) for o in outs], axis=1)
    return np.ascontiguousarray(rows.reshape(cfg.B, cfg.T, cfg.C))


# =====================================================================
# Public entry point
# =====================================================================

_NC_CACHE = {}


def get_nc(debug=False):
    key = ("full", debug)
    if key not in _NC_CACHE:
        _NC_CACHE[key] = build_nc(FULL, debug=debug)
    return _NC_CACHE[key]


def kernel(x, w_qkv, w_proj):
    from concourse.bass_utils import run_bass_kernel_spmd
    nc = get_nc()
    in_maps = make_in_maps(x, w_qkv, w_proj)
    res = run_bass_kernel_spmd(nc, in_maps, list(range(N_CORES)))
    return gather([res.results[c]["out"] for c in range(N_CORES)])



# revision 35
# speedup vs baseline: 1.5726x; 1.0286x over previous
"""Self-contained Trainium2 Bass kernel for nn_MultiHeadAttention_71528385347884.

Strategy: head tensor-parallel across 8 cores (2 heads/core). Per core:
  - QKV projection with x transposed (feature-major q/k, token-major v)
  - RoPE via host-side A/B weight-column packing (no cross-partition ops)
  - causal attention in [s,t] score layout, softmax without max-subtraction
    (scores are bounded ~|4.5|), denominator via all-ones matmul
  - output projection exploits the reference's scrambled
    transpose(0,2,1,3).reshape(B,T,C): each core produces disjoint output
    rows -> host gather is pure concatenation.
"""

import math
import numpy as np
import ml_dtypes

# ---- problem constants (hardcoded; kernel.py must not read spec/reference) ----
B = 2
T = 2048          # sequence length per batch
C = 2048          # model dim
Dh = 128          # head dim
N_HEAD = 16
N_CORES = 8
H_LOCAL = 2       # heads per core
ROPE_BASE = 10000.0
SCALE = 1.0 / math.sqrt(Dh)

BF16 = ml_dtypes.bfloat16


class Cfg:
    """Size parameters so the same builder runs a small CoreSim config."""

    def __init__(self, B=B, T=T, C=C):
        assert T % 512 == 0 and C % 128 == 0
        self.B = B
        self.T = T
        self.C = C
        self.NCC = C // 128        # contraction chunks for qkv matmuls
        self.BT = B * T
        self.NT = T // 512         # 512-wide t-tiles per batch
        self.GRP = C // Dh         # tokens folded per output row by the reshape
        self.TAU = T // self.GRP   # output rows per (b, h); must be 128
        assert self.TAU == 128
        self.ET = max(1, C // 512)  # 512-wide e-tiles of the output
        self.JQK = 4 * 128         # qA,qB,kA,kB feature blocks
        self.JV = H_LOCAL * 128


FULL = Cfg()


# =====================================================================
# Device program builder
# =====================================================================

def build_nc(cfg: Cfg, debug=False, repeat=1, phases="BCD", loop_repeat=None,
             fine=False, lag=2, prbufs=7):
    import contextlib
    import concourse.bass as bass
    import concourse.mybir as mybir
    import concourse.tile as tile
    from concourse import bacc

    f32 = mybir.dt.float32
    bf16 = mybir.dt.bfloat16
    Exp = mybir.ActivationFunctionType.Exp
    Copy = mybir.ActivationFunctionType.Copy

    nc = bacc.Bacc(None, target_bir_lowering=False, debug=debug)

    xt_d = nc.dram_tensor("xt", [128, cfg.NCC, cfg.BT], bf16, kind="ExternalInput")
    wqk_d = nc.dram_tensor("wqk", [128, cfg.NCC, cfg.JQK], bf16, kind="ExternalInput")
    wv_d = nc.dram_tensor("wv", [128, cfg.NCC, cfg.JV], bf16, kind="ExternalInput")
    wp_d = nc.dram_tensor("wp", [128, cfg.GRP, cfg.C], bf16, kind="ExternalInput")
    cc2_d = nc.dram_tensor("cc2", [128, cfg.T], bf16, kind="ExternalInput")
    spm_d = nc.dram_tensor("spm", [128, cfg.T], bf16, kind="ExternalInput")
    smp_d = nc.dram_tensor("smp", [128, cfg.T], bf16, kind="ExternalInput")
    masks_d = nc.dram_tensor("masks", [128, 2, 128], bf16,
                             kind="ExternalInput")
    # repeat>1 (bench-only) gets a per-rep output slice so no rep's stores
    # are dead — guards the marginal-rep timing against compiler DCE.
    out_d = nc.dram_tensor("out", [repeat, cfg.B, H_LOCAL, 128, cfg.C], f32,
                           kind="ExternalOutput")

    with tile.TileContext(nc) as tc:
        with tc.tile_pool(name="persist", bufs=1) as persist:
            # ---- persistent SBUF state ----
            wqk_sb = persist.tile([128, cfg.NCC, cfg.JQK], bf16, name="wqk_sb",
                                  tag="wqk_sb")
            wv_sb = persist.tile([128, cfg.NCC, cfg.JV], bf16, name="wv_sb",
                                 tag="wv_sb")
            cc2_sb = persist.tile([128, cfg.T], bf16, name="cc2_sb", tag="cc2_sb")
            spm_sb = persist.tile([128, cfg.T], bf16, name="spm_sb", tag="spm_sb")
            smp_sb = persist.tile([128, cfg.T], bf16, name="smp_sb", tag="smp_sb")
            masks_sb = persist.tile([128, 2, 128], bf16, name="masks_sb",
                                    tag="masks_sb")
            ones_sb = persist.tile([128, 128], bf16, name="ones_sb", tag="ones_sb")

            nc.vector.memset(ones_sb[:], 1.0)

            # per-(b, head-or-tile) persistent tensors; q/k are stored
            # head-contiguous ([dims 0:128 of head h] on partitions) so the
            # score matmuls contract K=128 in one shot.
            qh_sb, kh_sb = {}, {}
            v_sb, attn_sb = {}, {}
            for b in range(cfg.B):
                for hl in range(H_LOCAL):
                    qh_sb[(b, hl)] = persist.tile([128, cfg.T], bf16,
                                                  name=f"qh_{b}_{hl}",
                                                  tag=f"qh_{b}_{hl}")
                    kh_sb[(b, hl)] = persist.tile([128, cfg.T], bf16,
                                                  name=f"kh_{b}_{hl}",
                                                  tag=f"kh_{b}_{hl}")
                for hl in range(H_LOCAL):
                    v_sb[(b, hl)] = persist.tile(
                        [128, cfg.T // 128, 128], bf16,
                        name=f"v_{b}_{hl}", tag=f"v_{b}_{hl}")
                    attn_sb[(b, hl)] = persist.tile(
                        [128, cfg.T], bf16,
                        name=f"at_{b}_{hl}", tag=f"at_{b}_{hl}")


            _loop_es = contextlib.ExitStack()
            if loop_repeat:
                # steady-state timing mode: weights/trig loaded in a
                # prologue, then For_i repeats one rep body on-device.
                nc.scalar.dma_start(wqk_sb[:], wqk_d[:])
                nc.scalar.dma_start(wv_sb[:], wv_d[:])
                nc.scalar.dma_start(cc2_sb[:], cc2_d[:])
                nc.scalar.dma_start(spm_sb[:], spm_d[:])
                nc.scalar.dma_start(smp_sb[:], smp_d[:])
                nc.scalar.dma_start(masks_sb[:], masks_d[:])
                if "B" not in phases:
                    for b in range(cfg.B):
                        for hl in range(H_LOCAL):
                            nc.vector.memset(qh_sb[(b, hl)][:], 0.01)
                            nc.vector.memset(kh_sb[(b, hl)][:], 0.01)
                            nc.vector.memset(v_sb[(b, hl)][:], 0.01)
                if "D" in phases and "C" not in phases:
                    for b in range(cfg.B):
                        for hl in range(H_LOCAL):
                            nc.vector.memset(attn_sb[(b, hl)][:], 0.01)
                _loop_es.enter_context(tc.For_i(0, loop_repeat))

            for rep in range(repeat):
                # Phase plan (PE keeps busy through attention's exp waits):
                #   B(b0) -> C(b0) interleaved with B(b1) -> C(b1)
                #   interleaved with D(b0) -> D(b1).
                # One unified PSUM pool, 8 banks exactly:
                #   pj [128,512]x2 (B qkv groups, D proj groups)
                #   ps [128,2,512]x2 (C scores + ones-denominator)
                #   po [128,512]x2 (C attn accumulators)
                with (
                    tc.tile_pool(name=f"xb_pool{rep}", bufs=4) as xb_pool,
                    tc.tile_pool(name=f"rtmp{rep}", bufs=4) as rtmp,
                    tc.tile_pool(name=f"probs{rep}", bufs=prbufs) as probs_pool,
                    tc.tile_pool(name=f"ssum{rep}", bufs=2) as ssum_pool,
                    tc.tile_pool(name=f"rec{rep}", bufs=2) as rec_pool,
                    tc.tile_pool(name=f"ostg{rep}", bufs=3) as ostg_pool,
                    tc.tile_pool(name=f"vfm{rep}", bufs=4) as vfm_pool,
                    tc.tile_pool(name=f"ps{rep}", bufs=2, space="PSUM") as psum,
                ):
                    half = cfg.NCC // 2
                    qtr = max(1, half // 2)
                    xtiles = {}

                    def fetch_x(b, tt, chunked=False):
                        bt0 = b * cfg.T + tt * 512
                        xlo = xb_pool.tile([128, half, 512], bf16,
                                           name=f"xbl_{b}_{tt}", tag="xb")
                        xhi = xb_pool.tile([128, half, 512], bf16,
                                           name=f"xbh_{b}_{tt}", tag="xb")
                        if chunked:
                            # startup: single-chunk DMAs for the first 4
                            # (fast first matmul), 2-chunk after (halve the
                            # ~0.6us-per-op issue load on the queues).
                            steps = [(0, 1), (1, 1), (2, 2),
                                     (4, 4), (8, 8)]
                            for c, w in steps:
                                xdst = (xlo if c < half else xhi)
                                par = (c // w) % 2 == 0
                                xq = (nc.sync if par else nc.gpsimd)
                                wq = (nc.gpsimd if par else nc.sync)
                                nc.scalar.dma_start(wqk_sb[:, c:c + w, :],
                                                    wqk_d[:, c:c + w, :])
                                wq.dma_start(wv_sb[:, c:c + w, :],
                                             wv_d[:, c:c + w, :])
                                cl = c % half
                                xq.dma_start(
                                    xdst[:, cl:cl + w, :],
                                    xt_d[:, c:c + w, bt0:bt0 + 512])
                            nc.scalar.dma_start(cc2_sb[:, 0:512],
                                                cc2_d[:, 0:512])
                            nc.scalar.dma_start(spm_sb[:, 0:512],
                                                spm_d[:, 0:512])
                            nc.scalar.dma_start(smp_sb[:, 0:512],
                                                smp_d[:, 0:512])
                        else:
                            nc.sync.dma_start(xlo[:, 0:qtr, :],
                                              xt_d[:, 0:qtr, bt0:bt0 + 512])
                            nc.gpsimd.dma_start(xlo[:, qtr:half, :],
                                                xt_d[:, qtr:half,
                                                     bt0:bt0 + 512])
                            nc.sync.dma_start(xhi[:, 0:qtr, :],
                                              xt_d[:, half:half + qtr,
                                                   bt0:bt0 + 512])
                            nc.gpsimd.dma_start(xhi[:, qtr:half, :],
                                                xt_d[:, half + qtr:cfg.NCC,
                                                     bt0:bt0 + 512])
                        xtiles[(b, tt)] = (xlo, xhi)

                    def xb_of(b, tt, ccs):
                        xlo, xhi = xtiles[(b, tt)]
                        return (xlo if ccs < half else xhi)[:, ccs % half, :]

                    def rope_pair(b, tt, Aps, Bps, d0, d1):
                        # rotA = A*C2 + B*S+-,  rotB = B*C2 + A*S-+
                        # rotA rows 0:64 -> d0[0:64]; rows 64:128 -> d1[64:]
                        # rotB rows 0:64 -> d0[64:]; rows 64:128 -> d1[0:64]
                        # flat 2D APs throughout: 3D / partial-partition APs
                        # run 3-8x slower on ACT/DVE (HW slow path).
                        tl = slice(tt * 512, (tt + 1) * 512)
                        a2 = rtmp.tile([128, 512], bf16, name="a2",
                                       tag="ab", bufs=4)
                        b2 = rtmp.tile([128, 512], bf16, name="b2",
                                       tag="ab", bufs=4)
                        nc.scalar.activation(a2[:], Aps, Copy)
                        nc.scalar.activation(b2[:], Bps, Copy)
                        m1 = rtmp.tile([128, 512], bf16, name="m1", tag="rt")
                        m2 = rtmp.tile([128, 512], bf16, name="m2", tag="rt")
                        m3 = rtmp.tile([128, 512], bf16, name="m3", tag="rt")
                        m4 = rtmp.tile([128, 512], bf16, name="m4", tag="rt")
                        nc.vector.tensor_mul(m1[:], a2[:], cc2_sb[:, tl])
                        nc.vector.tensor_mul(m2[:], b2[:], spm_sb[:, tl])
                        nc.vector.tensor_mul(m3[:], b2[:], cc2_sb[:, tl])
                        nc.vector.tensor_mul(m4[:], a2[:], smp_sb[:, tl])
                        nc.vector.tensor_add(d0[0:64, tl],
                                             m1[0:64, :], m2[0:64, :])
                        nc.vector.tensor_add(d1[64:128, tl],
                                             m1[64:128, :], m2[64:128, :])
                        rb = rtmp.tile([128, 512], bf16, name="rb", tag="rtb",
                                       bufs=2)
                        nc.vector.tensor_add(rb[:], m3[:], m4[:])
                        nc.gpsimd.dma_start(d0[64:128, tl], rb[0:64, :])
                        nc.gpsimd.dma_start(d1[0:64, tl], rb[64:128, :])

                    def v_finish(b, tt, hl, pv):
                        vf = vfm_pool.tile([128, 512], bf16,
                                           name=f"vf_{b}_{tt}_{hl}", tag="vf")
                        nc.scalar.activation(vf[:], pv, Copy)
                        nc.sync.dma_start_transpose(
                            v_sb[(b, hl)][:, tt * 4:(tt + 1) * 4, :], vf[:])

                    def b_tile_wide(b, tt):
                        """chunk-major qkv tile: 6 psum groups at once
                        (pj x2 + ps halves + po x2)."""
                        pjA = psum.tile([128, 512], f32, name=f"bqA_{b}_{tt}",
                                        tag="pj")
                        pjB = psum.tile([128, 512], f32, name=f"bqB_{b}_{tt}",
                                        tag="pj")
                        phk = psum.tile([128, 2, 512], f32,
                                        name=f"bk_{b}_{tt}", tag="ps")
                        pv = [psum.tile([128, 512], f32, name=f"bv_{b}_{tt}_{hl}",
                                        tag="po") for hl in range(2)]
                        for ccs in range(cfg.NCC):
                            xb = xb_of(b, tt, ccs)
                            st = (ccs == 0)
                            sp = (ccs == cfg.NCC - 1)
                            nc.tensor.matmul(pjA[:], wqk_sb[:, ccs, 0:128],
                                             xb, start=st, stop=sp)
                            nc.tensor.matmul(pjB[:], wqk_sb[:, ccs, 128:256],
                                             xb, start=st, stop=sp)
                            nc.tensor.matmul(phk[:, 0, :],
                                             wqk_sb[:, ccs, 256:384],
                                             xb, start=st, stop=sp)
                            nc.tensor.matmul(phk[:, 1, :],
                                             wqk_sb[:, ccs, 384:512],
                                             xb, start=st, stop=sp)
                            for hl in range(2):
                                nc.tensor.matmul(
                                    pv[hl][:],
                                    wv_sb[:, ccs, hl * 128:(hl + 1) * 128],
                                    xb, start=st, stop=sp)
                        rope_pair(b, tt, pjA[:], pjB[:],
                                  qh_sb[(b, 0)], qh_sb[(b, 1)])
                        rope_pair(b, tt, phk[:, 0, :], phk[:, 1, :],
                                  kh_sb[(b, 0)], kh_sb[(b, 1)])
                        for hl in range(2):
                            v_finish(b, tt, hl, pv[hl][:])

                    def b_tile_units(b, tt):
                        """j-major qkv tile as a list of closures, each
                        holding at most 2 psum banks (interleavable with C)."""
                        units = [lambda b=b, tt=tt: fetch_x(b, tt)]
                        state = {}

                        # nparts>1 splits each 16-chunk chain into nparts
                        # emission subunits (same instructions, finer
                        # interleave granularity vs C blocks).
                        nparts = 4 if fine else 1
                        step = cfg.NCC // nparts

                        def jgroup(key, w_sb):
                            subs = []
                            for p0 in range(0, cfg.NCC, step):
                                def f(p0=p0, key=key, w_sb=w_sb):
                                    if p0 == 0:
                                        state[key] = psum.tile(
                                            [128, 512], f32,
                                            name=f"u{key}_{b}_{tt}",
                                            tag="pj")
                                    p = state[key][:]
                                    for ccs in range(p0, p0 + step):
                                        nc.tensor.matmul(
                                            p, w_sb(ccs), xb_of(b, tt, ccs),
                                            start=(ccs == 0),
                                            stop=(ccs == cfg.NCC - 1))
                                subs.append(f)
                            return subs

                        units.extend(jgroup("qA",
                                     lambda c: wqk_sb[:, c, 0:128]))
                        units.extend(jgroup("qB",
                                     lambda c: wqk_sb[:, c, 128:256]))
                        units.append(lambda: rope_pair(
                            b, tt, state["qA"][:], state["qB"][:],
                            qh_sb[(b, 0)], qh_sb[(b, 1)]))

                        units.extend(jgroup("kA",
                                     lambda c: wqk_sb[:, c, 256:384]))
                        units.extend(jgroup("kB",
                                     lambda c: wqk_sb[:, c, 384:512]))
                        units.append(lambda: rope_pair(
                            b, tt, state["kA"][:], state["kB"][:],
                            kh_sb[(b, 0)], kh_sb[(b, 1)]))

                        for hl in range(2):
                            for p0 in range(0, cfg.NCC, step):
                                def vv(hl=hl, p0=p0):
                                    key = f"v{hl}"
                                    if p0 == 0:
                                        state[key] = psum.tile(
                                            [128, 512], f32,
                                            name=f"uv_{b}_{tt}_{hl}",
                                            tag="pj")
                                    pvt = state[key]
                                    for ccs in range(p0, p0 + step):
                                        nc.tensor.matmul(
                                            pvt[:],
                                            wv_sb[:, ccs,
                                                  hl * 128:(hl + 1) * 128],
                                            xb_of(b, tt, ccs),
                                            start=(ccs == 0),
                                            stop=(ccs == cfg.NCC - 1))
                                    if p0 + step == cfg.NCC:
                                        v_finish(b, tt, hl, pvt[:])
                                units.append(vv)
                        return units

                    # ---------------- attention (phase C) -----------------
                    def make_c_blocks(b):
                        blocks = []
                        for tt in range(cfg.NT):
                            n_sc = (tt + 1) * 4
                            state = {}

                            def emit_po(entry, b=b, tt=tt, n_sc=n_sc,
                                        state=state):
                                sc_, off_, pr_ = entry
                                for h in range(2):
                                    nc.tensor.matmul(
                                        state["po"][h][:, off_:],
                                        v_sb[(b, h)][:, sc_, :],
                                        pr_[:, h, off_:],
                                        start=(sc_ == 0),
                                        stop=(sc_ == n_sc - 1))

                            def block(sc, b=b, tt=tt, n_sc=n_sc,
                                      state=state, emit_po=emit_po):
                                def f():
                                    if sc == 0:
                                        state["po"] = [
                                            psum.tile([128, 512], f32,
                                                      name=f"po_{b}_{tt}_{h}",
                                                      tag="po")
                                            for h in range(2)]
                                        state["ssum"] = ssum_pool.tile(
                                            [128, 2, 512], bf16,
                                            name=f"ss_{b}_{tt}", tag="ss")
                                        state["prq"] = []
                                    off = max(0, (sc - tt * 4) * 128)
                                    sl = slice(sc * 128, (sc + 1) * 128)
                                    qsl = slice(tt * 512 + off,
                                                (tt + 1) * 512)
                                    ph = psum.tile([128, 2, 512], f32,
                                                   name=f"ps_{b}_{tt}_{sc}",
                                                   tag="ps")
                                    for h in range(2):
                                        nc.tensor.matmul(
                                            ph[:, h, off:],
                                            kh_sb[(b, h)][:, sl],
                                            qh_sb[(b, h)][:, qsl],
                                            start=True, stop=True)
                                    pr = probs_pool.tile([128, 2, 512], bf16,
                                                         name="pr", tag="pr")
                                    nc.scalar.activation(pr[:, :, off:],
                                                         ph[:, :, off:], Exp,
                                                         scale=SCALE)
                                    if sc >= tt * 4:
                                        nc.vector.tensor_mul(
                                            pr[:, :, off:off + 128],
                                            pr[:, :, off:off + 128],
                                            masks_sb[:])
                                    ssum = state["ssum"]
                                    if sc == 0:
                                        nc.vector.tensor_copy(ssum[:], pr[:])
                                    else:
                                        nc.vector.tensor_add(
                                            ssum[:, :, off:],
                                            ssum[:, :, off:],
                                            pr[:, :, off:])
                                    state["prq"].append((sc, off, pr))
                                    if len(state["prq"]) > lag:
                                        emit_po(state["prq"].pop(0))
                                return f

                            def tail(b=b, tt=tt, state=state,
                                     emit_po=emit_po):
                                while state["prq"]:
                                    emit_po(state["prq"].pop(0))
                                pd = psum.tile([128, 2, 512], f32,
                                               name=f"pd_{b}_{tt}", tag="ps")
                                ssum = state["ssum"]
                                for h in range(2):
                                    nc.tensor.matmul(pd[:, h, :], ones_sb[:],
                                                     ssum[:, h, :],
                                                     start=True, stop=True)
                                for h in range(2):
                                    rec = rec_pool.tile([128, 512], f32,
                                                        name=f"rec_{h}",
                                                        tag="rec")
                                    nc.vector.reciprocal(rec[:], pd[:, h, :])
                                    nc.vector.tensor_mul(
                                        attn_sb[(b, h)][:,
                                                        tt * 512:(tt + 1) * 512],
                                        state["po"][h][:], rec[:])

                            blocks.extend(block(sc) for sc in range(n_sc))
                            blocks.append(tail)
                        return blocks

                    # ---------------- output projection (phase D) ----------
                    # wpe is batch-independent: cache 256-wide e-slices
                    # ([128,16,256] = 8KB each, 4 live). b1 runs e-order
                    # 4,5,6,7,0,1,2,3: its first four units hit still-live
                    # tiles and the lookahead prefetch hides the reloads.
                    WPE_BUFS = 4
                    NE8 = cfg.C // 256
                    wpe_tiles = {}
                    wpe_fifo = []
                    wpe_cnt = [0]

                    d_seq = ([(0, e) for e in range(NE8)] +
                             [(1, e) for e in
                              list(range(NE8 // 2, NE8)) +
                              list(range(NE8 // 2))])

                    def issue_wpe_e(e):
                        wpe_cnt[0] += 1
                        w = persist.tile([128, cfg.GRP, 256], bf16,
                                         name=f"wpe_{rep}_{wpe_cnt[0]}",
                                         tag="wpe", bufs=WPE_BUFS)
                        for g0 in range(0, cfg.GRP, 8):
                            nc.scalar.dma_start(
                                w[:, g0:g0 + 8, :],
                                wp_d[:, g0:g0 + 8, e * 256:(e + 1) * 256])
                        wpe_tiles[e] = w
                        wpe_fifo.append(e)
                        if len(wpe_fifo) > WPE_BUFS:
                            del wpe_tiles[wpe_fifo.pop(0)]

                    def wpe_prefetch(from_idx, lookahead=3):
                        """Issue tiles for upcoming pairs; never evict a tile
                        a not-yet-emitted pair still needs."""
                        protect = {e for (_bb, e) in d_seq[from_idx:]
                                   if e in wpe_tiles}
                        for j in range(from_idx,
                                       min(from_idx + lookahead, len(d_seq))):
                            e = d_seq[j][1]
                            if e in wpe_tiles:
                                continue
                            if (len(wpe_fifo) >= WPE_BUFS
                                    and wpe_fifo[0] in protect):
                                break
                            issue_wpe_e(e)
                            protect.add(e)

                    def d_unit_parts(b, e, hl, nparts=4):
                        """d_unit split into nparts emission subunits."""
                        dstate = {}
                        dstep = cfg.GRP // nparts
                        subs = []
                        for p0 in range(0, cfg.GRP, dstep):
                            def f(p0=p0):
                                if p0 == 0:
                                    if e not in wpe_tiles:
                                        issue_wpe_e(e)
                                    dstate["pp"] = psum.tile(
                                        [128, 256], f32,
                                        name=f"pp_{b}_{e}_{hl}", tag="pj")
                                    dstate["w"] = wpe_tiles[e]
                                pp, w = dstate["pp"], dstate["w"]
                                at = attn_sb[(b, hl)]
                                for u in range(p0, p0 + dstep):
                                    nc.tensor.matmul(
                                        pp[:], at[:, u::cfg.GRP], w[:, u, :],
                                        start=(u == 0),
                                        stop=(u == cfg.GRP - 1))
                                if p0 + dstep == cfg.GRP:
                                    el = slice(e * 256, (e + 1) * 256)
                                    stg = ostg_pool.tile(
                                        [128, 256], f32,
                                        name=f"stg_{b}_{e}_{hl}", tag="stg")
                                    nc.vector.tensor_copy(stg[:], pp[:])
                                    oq = (nc.sync if (e + hl) % 2 == 0
                                          else nc.gpsimd)
                                    oq.dma_start(out_d[rep, b, hl, :, el],
                                                 stg[:])
                                    wpe_prefetch(d_seq.index((b, e)) + 1)
                            subs.append(f)
                        return subs

                    def d_unit(b, e, hl, split=1):
                        def f():
                            idx = d_seq.index((b, e))
                            if e not in wpe_tiles:
                                issue_wpe_e(e)
                            w = wpe_tiles[e]
                            at = attn_sb[(b, hl)]
                            ew = 256 // split
                            for sp in range(split):
                                el = slice(e * 256 + sp * ew,
                                           e * 256 + (sp + 1) * ew)
                                wl = slice(sp * ew, (sp + 1) * ew)
                                pp = psum.tile([128, ew], f32,
                                               name=f"pp_{b}_{e}_{hl}_{sp}",
                                               tag="pj")
                                for u in range(cfg.GRP):
                                    nc.tensor.matmul(pp[:],
                                                     at[:, u::cfg.GRP],
                                                     w[:, u, wl],
                                                     start=(u == 0),
                                                     stop=(u == cfg.GRP - 1))
                                stg = ostg_pool.tile([128, ew], f32,
                                                     name=f"stg_{b}_{e}_{hl}",
                                                     tag="stg")
                                # DVE copy: ACT is exp-saturated in segment2,
                                # and a queued stg copy there delays the pj
                                # psum rotation (head-of-line).
                                nc.vector.tensor_copy(stg[:], pp[:])
                                oq = (nc.sync if (e + hl + sp) % 2 == 0
                                      else nc.gpsimd)
                                oq.dma_start(out_d[rep, b, hl, :, el],
                                             stg[:])
                            wpe_prefetch(idx + 1)
                        return f

                    def interleave(blocks, units, margin=4):
                        n = max(1, len(blocks) - margin)
                        per = len(units) / n
                        acc = 0.0
                        units = list(units)
                        for i, bl in enumerate(blocks):
                            bl()
                            if i < n:
                                acc += per
                            while acc >= 1.0 and units:
                                units.pop(0)()
                                acc -= 1.0
                        for u in units:
                            u()

                    # ------------------- emission order --------------------
                    first = (rep == 0) and loop_repeat is None

                    def dummy_out(src_tile):
                        stg = ostg_pool.tile([128, 512], f32,
                                             name="dstg", tag="stg")
                        nc.vector.tensor_copy(stg[:], src_tile)
                        nc.sync.dma_start(out_d[rep, 0, 0, :, 0:512], stg[:])

                    if "B" in phases:
                        fetch_x(0, 0, chunked=first)
                        fetch_x(0, 1)
                        b_tile_wide(0, 0)
                        fetch_x(0, 2)
                        if first:
                            # emitted here so their ACT-queue issue (and hence
                            # the transfers) trail tile-0's drain copies
                            # instead of stealing DMA bandwidth from its
                            # chunk train.
                            if cfg.T > 512:
                                nc.scalar.dma_start(cc2_sb[:, 512:cfg.T],
                                                    cc2_d[:, 512:cfg.T])
                                nc.scalar.dma_start(spm_sb[:, 512:cfg.T],
                                                    spm_d[:, 512:cfg.T])
                                nc.scalar.dma_start(smp_sb[:, 512:cfg.T],
                                                    smp_d[:, 512:cfg.T])
                            nc.scalar.dma_start(masks_sb[:], masks_d[:])
                        b_tile_wide(0, 1)
                        fetch_x(0, 3)
                        b_tile_wide(0, 2)
                        b_tile_wide(0, 3)
                    elif first:
                        nc.scalar.dma_start(masks_sb[:], masks_d[:])

                    if phases == "BCD":
                        wpe_prefetch(0, lookahead=2)
                        b1_units = []
                        for tt in range(cfg.NT):
                            b1_units.extend(b_tile_units(1, tt))
                        interleave(make_c_blocks(0), b1_units, margin=6)

                        if fine:
                            d0_units = []
                            for e in range(NE8):
                                for hl in range(2):
                                    d0_units.extend(d_unit_parts(0, e, hl))
                        else:
                            d0_units = [d_unit(0, e, hl)
                                        for e in range(NE8)
                                        for hl in range(2)]
                        # margin=1: d0 fillers (pj tag only — no po conflict)
                        # run right through C(b1)'s tail, covering the
                        # attn-mul drain that gates D(b1)'s first LDW.
                        interleave(make_c_blocks(1), d0_units, margin=-9)

                        for _b1, e in d_seq[NE8:]:
                            for hl in range(2):
                                d_unit(1, e, hl)()
                    else:
                        if "B" in phases:
                            b1_units = []
                            for tt in range(cfg.NT):
                                b1_units.extend(b_tile_units(1, tt))
                            if "C" in phases:
                                interleave(make_c_blocks(0), b1_units,
                                           margin=6)
                            else:
                                for u in b1_units:
                                    u()
                        elif "C" in phases:
                            for bl in make_c_blocks(0):
                                bl()
                        if "C" in phases:
                            for bl in make_c_blocks(1):
                                bl()
                        if "D" in phases:
                            wpe_prefetch(0, lookahead=2)
                            for b in range(cfg.B):
                                for e in range(NE8):
                                    for hl in range(2):
                                        d_unit(b, e, hl)()
                        elif "C" in phases:
                            dummy_out(attn_sb[(1, 1)][:, 0:512])
                        else:
                            dummy_out(qh_sb[(1, 0)][:, 0:512])

            _loop_es.close()

    nc.compile()
    return nc


# =====================================================================
# Host-side input prep / output gather
# =====================================================================

def _part_major(a2d, ncc):
    """[ncc*128, F] -> [128, ncc, F] with row r = chunk*128 + p."""
    F = a2d.shape[1]
    return np.ascontiguousarray(
        a2d.reshape(ncc, 128, F).transpose(1, 0, 2))


def make_trig(cfg: Cfg):
    pos = np.arange(cfg.T, dtype=np.float64)[None, :]        # [1,T]
    j = np.arange(64, dtype=np.float64)[:, None]             # [64,1]
    inv = ROPE_BASE ** (-2.0 * j / Dh)
    ang = pos * inv                                          # [64,T]
    sin = np.sin(ang).astype(np.float32)
    cos = np.cos(ang).astype(np.float32)
    cc2 = np.concatenate([cos, cos], axis=0).astype(BF16)    # [128,T]
    spm = np.concatenate([-sin, sin], axis=0).astype(BF16)
    smp = np.concatenate([sin, -sin], axis=0).astype(BF16)
    return cc2, spm, smp


def make_masks():
    # one lower-triangular 128x128 block mask, duplicated for the 2 heads:
    # within a diagonal block the valid region is local col j >= partition p.
    p = np.arange(128)[:, None]
    jj = np.arange(128)[None, :]
    tri = (p <= jj)
    return np.stack([tri, tri], axis=1).astype(BF16)         # [128,2,128]


def make_in_maps(x, w_qkv, w_proj, cfg: Cfg = FULL, n_cores=N_CORES,
                 n_head=N_HEAD):
    x = np.asarray(x, np.float32)
    w_qkv = np.asarray(w_qkv, np.float32)
    w_proj = np.asarray(w_proj, np.float32)
    Cm = cfg.C

    xT = np.ascontiguousarray(x.reshape(cfg.BT, Cm).T)       # [C, BT]
    xt = _part_major(xT, cfg.NCC).astype(BF16)
    wp = _part_major(w_proj, cfg.GRP).astype(BF16)
    cc2, spm, smp = make_trig(cfg)
    masks = make_masks()

    wq = w_qkv[:, 0:Cm]
    wk = w_qkv[:, Cm:2 * Cm]
    wv_all = w_qkv[:, 2 * Cm:3 * Cm]

    in_maps = []
    for c in range(n_cores):
        h0, h1 = 2 * c, 2 * c + 1
        q0 = wq[:, h0 * 128:(h0 + 1) * 128]
        q1 = wq[:, h1 * 128:(h1 + 1) * 128]
        k0 = wk[:, h0 * 128:(h0 + 1) * 128]
        k1 = wk[:, h1 * 128:(h1 + 1) * 128]
        qA = np.concatenate([q0[:, 0:64], q1[:, 64:128]], axis=1)
        qB = np.concatenate([q0[:, 64:128], q1[:, 0:64]], axis=1)
        kA = np.concatenate([k0[:, 0:64], k1[:, 64:128]], axis=1)
        kB = np.concatenate([k0[:, 64:128], k1[:, 0:64]], axis=1)
        wqk = _part_major(
            np.concatenate([qA, qB, kA, kB], axis=1), cfg.NCC).astype(BF16)
        wv = _part_major(
            np.concatenate([wv_all[:, h0 * 128:(h0 + 1) * 128],
                            wv_all[:, h1 * 128:(h1 + 1) * 128]], axis=1),
            cfg.NCC).astype(BF16)
        in_maps.append(dict(xt=xt, wqk=wqk, wv=wv, wp=wp,
                            cc2=cc2, spm=spm, smp=smp, masks=masks))
    return in_maps


def gather(outs, cfg: Cfg = FULL):
    """outs: per-core [rep, B, H_LOCAL, 128, C] -> full [B, T, C]."""
    rows = np.concatenate(
        [o[-1].reshape(cfg.B, H_LOCAL * 128, cfg.C) for o in outs], axis=1)
    return np.ascontiguousarray(rows.reshape(cfg.B, cfg.T, cfg.C))


# =====================================================================
# Public entry point
# =====================================================================

_NC_CACHE = {}


def get_nc(debug=False):
    key = ("full", debug)
    if key not in _NC_CACHE:
        _NC_CACHE[key] = build_nc(FULL, debug=debug)
    return _NC_CACHE[key]


def kernel(x, w_qkv, w_proj):
    from concourse.bass_utils import run_bass_kernel_spmd
    nc = get_nc()
    in_maps = make_in_maps(x, w_qkv, w_proj)
    res = run_bass_kernel_spmd(nc, in_maps, list(range(N_CORES)))
    return gather([res.results[c]["out"] for c in range(N_CORES)])



# revision 38
# speedup vs baseline: 1.5780x; 1.0035x over previous
"""Self-contained Trainium2 Bass kernel for nn_MultiHeadAttention_71528385347884.

Strategy: head tensor-parallel across 8 cores (2 heads/core). Per core:
  - QKV projection with x transposed (feature-major q/k, token-major v)
  - RoPE via host-side A/B weight-column packing (no cross-partition ops)
  - causal attention in [s,t] score layout, softmax without max-subtraction
    (scores are bounded ~|4.5|), denominator via all-ones matmul
  - output projection exploits the reference's scrambled
    transpose(0,2,1,3).reshape(B,T,C): each core produces disjoint output
    rows -> host gather is pure concatenation.
"""

import math
import numpy as np
import ml_dtypes

# ---- problem constants (hardcoded; kernel.py must not read spec/reference) ----
B = 2
T = 2048          # sequence length per batch
C = 2048          # model dim
Dh = 128          # head dim
N_HEAD = 16
N_CORES = 8
H_LOCAL = 2       # heads per core
ROPE_BASE = 10000.0
SCALE = 1.0 / math.sqrt(Dh)

BF16 = ml_dtypes.bfloat16


class Cfg:
    """Size parameters so the same builder runs a small CoreSim config."""

    def __init__(self, B=B, T=T, C=C):
        assert T % 512 == 0 and C % 128 == 0
        self.B = B
        self.T = T
        self.C = C
        self.NCC = C // 128        # contraction chunks for qkv matmuls
        self.BT = B * T
        self.NT = T // 512         # 512-wide t-tiles per batch
        self.GRP = C // Dh         # tokens folded per output row by the reshape
        self.TAU = T // self.GRP   # output rows per (b, h); must be 128
        assert self.TAU == 128
        self.ET = max(1, C // 512)  # 512-wide e-tiles of the output
        self.JQK = 4 * 128         # qA,qB,kA,kB feature blocks
        self.JV = H_LOCAL * 128


FULL = Cfg()


# =====================================================================
# Device program builder
# =====================================================================

def build_nc(cfg: Cfg, debug=False, repeat=1, phases="BCD", loop_repeat=None,
             fine=False, lag=2, prbufs=7, stg_pool=False, deep=False,
             m0=6, m1=-9):
    import contextlib
    import concourse.bass as bass
    import concourse.mybir as mybir
    import concourse.tile as tile
    from concourse import bacc

    f32 = mybir.dt.float32
    bf16 = mybir.dt.bfloat16
    Exp = mybir.ActivationFunctionType.Exp
    Copy = mybir.ActivationFunctionType.Copy

    nc = bacc.Bacc(None, target_bir_lowering=False, debug=debug)

    xt_d = nc.dram_tensor("xt", [128, cfg.NCC, cfg.BT], bf16, kind="ExternalInput")
    wqk_d = nc.dram_tensor("wqk", [128, cfg.NCC, cfg.JQK], bf16, kind="ExternalInput")
    wv_d = nc.dram_tensor("wv", [128, cfg.NCC, cfg.JV], bf16, kind="ExternalInput")
    wp_d = nc.dram_tensor("wp", [128, cfg.GRP, cfg.C], bf16, kind="ExternalInput")
    cc2_d = nc.dram_tensor("cc2", [128, cfg.T], bf16, kind="ExternalInput")
    spm_d = nc.dram_tensor("spm", [128, cfg.T], bf16, kind="ExternalInput")
    smp_d = nc.dram_tensor("smp", [128, cfg.T], bf16, kind="ExternalInput")
    masks_d = nc.dram_tensor("masks", [128, 2, 128], bf16,
                             kind="ExternalInput")
    # repeat>1 (bench-only) gets a per-rep output slice so no rep's stores
    # are dead — guards the marginal-rep timing against compiler DCE.
    out_d = nc.dram_tensor("out", [repeat, cfg.B, H_LOCAL, 128, cfg.C], f32,
                           kind="ExternalOutput")

    with tile.TileContext(nc) as tc:
        with tc.tile_pool(name="persist", bufs=1) as persist:
            # ---- persistent SBUF state ----
            wqk_sb = persist.tile([128, cfg.NCC, cfg.JQK], bf16, name="wqk_sb",
                                  tag="wqk_sb")
            wv_sb = persist.tile([128, cfg.NCC, cfg.JV], bf16, name="wv_sb",
                                 tag="wv_sb")
            cc2_sb = persist.tile([128, cfg.T], bf16, name="cc2_sb", tag="cc2_sb")
            spm_sb = persist.tile([128, cfg.T], bf16, name="spm_sb", tag="spm_sb")
            smp_sb = persist.tile([128, cfg.T], bf16, name="smp_sb", tag="smp_sb")
            masks_sb = persist.tile([128, 2, 128], bf16, name="masks_sb",
                                    tag="masks_sb")
            ones_sb = persist.tile([128, 128], bf16, name="ones_sb", tag="ones_sb")

            nc.vector.memset(ones_sb[:], 1.0)

            # per-(b, head-or-tile) persistent tensors; q/k are stored
            # head-contiguous ([dims 0:128 of head h] on partitions) so the
            # score matmuls contract K=128 in one shot.
            qh_sb, kh_sb = {}, {}
            v_sb, attn_sb = {}, {}
            for b in range(cfg.B):
                for hl in range(H_LOCAL):
                    qh_sb[(b, hl)] = persist.tile([128, cfg.T], bf16,
                                                  name=f"qh_{b}_{hl}",
                                                  tag=f"qh_{b}_{hl}")
                    kh_sb[(b, hl)] = persist.tile([128, cfg.T], bf16,
                                                  name=f"kh_{b}_{hl}",
                                                  tag=f"kh_{b}_{hl}")
                for hl in range(H_LOCAL):
                    v_sb[(b, hl)] = persist.tile(
                        [128, cfg.T // 128, 128], bf16,
                        name=f"v_{b}_{hl}", tag=f"v_{b}_{hl}")
                    attn_sb[(b, hl)] = persist.tile(
                        [128, cfg.T], bf16,
                        name=f"at_{b}_{hl}", tag=f"at_{b}_{hl}")


            _loop_es = contextlib.ExitStack()
            if loop_repeat:
                # steady-state timing mode: weights/trig loaded in a
                # prologue, then For_i repeats one rep body on-device.
                nc.scalar.dma_start(wqk_sb[:], wqk_d[:])
                nc.scalar.dma_start(wv_sb[:], wv_d[:])
                nc.scalar.dma_start(cc2_sb[:], cc2_d[:])
                nc.scalar.dma_start(spm_sb[:], spm_d[:])
                nc.scalar.dma_start(smp_sb[:], smp_d[:])
                nc.scalar.dma_start(masks_sb[:], masks_d[:])
                if "B" not in phases:
                    for b in range(cfg.B):
                        for hl in range(H_LOCAL):
                            nc.vector.memset(qh_sb[(b, hl)][:], 0.01)
                            nc.vector.memset(kh_sb[(b, hl)][:], 0.01)
                            nc.vector.memset(v_sb[(b, hl)][:], 0.01)
                if "D" in phases and "C" not in phases:
                    for b in range(cfg.B):
                        for hl in range(H_LOCAL):
                            nc.vector.memset(attn_sb[(b, hl)][:], 0.01)
                _loop_es.enter_context(tc.For_i(0, loop_repeat))

            for rep in range(repeat):
                # Phase plan (PE keeps busy through attention's exp waits):
                #   B(b0) -> C(b0) interleaved with B(b1) -> C(b1)
                #   interleaved with D(b0) -> D(b1).
                # One unified PSUM pool, 8 banks exactly:
                #   pj [128,512]x2 (B qkv groups, D proj groups)
                #   ps [128,2,512]x2 (C scores + ones-denominator)
                #   po [128,512]x2 (C attn accumulators)
                dd = 0  # SBUF exhausted; deep pools don't fit
                with (
                    tc.tile_pool(name=f"xb_pool{rep}", bufs=4) as xb_pool,
                    tc.tile_pool(name=f"rtmp{rep}", bufs=4 + dd) as rtmp,
                    tc.tile_pool(name=f"probs{rep}", bufs=prbufs) as probs_pool,
                    tc.tile_pool(name=f"ssum{rep}", bufs=2 + dd) as ssum_pool,
                    tc.tile_pool(name=f"rec{rep}", bufs=2 + dd) as rec_pool,
                    tc.tile_pool(name=f"ostg{rep}", bufs=3 + dd) as ostg_pool,
                    tc.tile_pool(name=f"vfm{rep}", bufs=4 + dd) as vfm_pool,
                    tc.tile_pool(name=f"ps{rep}", bufs=2, space="PSUM") as psum,
                ):
                    half = cfg.NCC // 2
                    qtr = max(1, half // 2)
                    xtiles = {}

                    def fetch_x(b, tt, chunked=False):
                        bt0 = b * cfg.T + tt * 512
                        xlo = xb_pool.tile([128, half, 512], bf16,
                                           name=f"xbl_{b}_{tt}", tag="xb")
                        xhi = xb_pool.tile([128, half, 512], bf16,
                                           name=f"xbh_{b}_{tt}", tag="xb")
                        if chunked:
                            # startup: single-chunk DMAs for the first 4
                            # (fast first matmul), 2-chunk after (halve the
                            # ~0.6us-per-op issue load on the queues).
                            steps = [(0, 1), (1, 1), (2, 2),
                                     (4, 4), (8, 8)]
                            for c, w in steps:
                                xdst = (xlo if c < half else xhi)
                                par = (c // w) % 2 == 0
                                xq = (nc.sync if par else nc.gpsimd)
                                wq = (nc.gpsimd if par else nc.sync)
                                nc.scalar.dma_start(wqk_sb[:, c:c + w, :],
                                                    wqk_d[:, c:c + w, :])
                                wq.dma_start(wv_sb[:, c:c + w, :],
                                             wv_d[:, c:c + w, :])
                                cl = c % half
                                xq.dma_start(
                                    xdst[:, cl:cl + w, :],
                                    xt_d[:, c:c + w, bt0:bt0 + 512])
                            nc.scalar.dma_start(cc2_sb[:, 0:512],
                                                cc2_d[:, 0:512])
                            nc.scalar.dma_start(spm_sb[:, 0:512],
                                                spm_d[:, 0:512])
                            nc.scalar.dma_start(smp_sb[:, 0:512],
                                                smp_d[:, 0:512])
                        else:
                            nc.sync.dma_start(xlo[:, 0:qtr, :],
                                              xt_d[:, 0:qtr, bt0:bt0 + 512])
                            nc.gpsimd.dma_start(xlo[:, qtr:half, :],
                                                xt_d[:, qtr:half,
                                                     bt0:bt0 + 512])
                            nc.sync.dma_start(xhi[:, 0:qtr, :],
                                              xt_d[:, half:half + qtr,
                                                   bt0:bt0 + 512])
                            nc.gpsimd.dma_start(xhi[:, qtr:half, :],
                                                xt_d[:, half + qtr:cfg.NCC,
                                                     bt0:bt0 + 512])
                        xtiles[(b, tt)] = (xlo, xhi)

                    def xb_of(b, tt, ccs):
                        xlo, xhi = xtiles[(b, tt)]
                        return (xlo if ccs < half else xhi)[:, ccs % half, :]

                    def rope_pair(b, tt, Aps, Bps, d0, d1):
                        # rotA = A*C2 + B*S+-,  rotB = B*C2 + A*S-+
                        # rotA rows 0:64 -> d0[0:64]; rows 64:128 -> d1[64:]
                        # rotB rows 0:64 -> d0[64:]; rows 64:128 -> d1[0:64]
                        # flat 2D APs throughout: 3D / partial-partition APs
                        # run 3-8x slower on ACT/DVE (HW slow path).
                        tl = slice(tt * 512, (tt + 1) * 512)
                        a2 = rtmp.tile([128, 512], bf16, name="a2",
                                       tag="ab", bufs=4)
                        b2 = rtmp.tile([128, 512], bf16, name="b2",
                                       tag="ab", bufs=4)
                        nc.scalar.activation(a2[:], Aps, Copy)
                        nc.scalar.activation(b2[:], Bps, Copy)
                        m1 = rtmp.tile([128, 512], bf16, name="m1", tag="rt")
                        m2 = rtmp.tile([128, 512], bf16, name="m2", tag="rt")
                        m3 = rtmp.tile([128, 512], bf16, name="m3", tag="rt")
                        m4 = rtmp.tile([128, 512], bf16, name="m4", tag="rt")
                        nc.vector.tensor_mul(m1[:], a2[:], cc2_sb[:, tl])
                        nc.vector.tensor_mul(m2[:], b2[:], spm_sb[:, tl])
                        nc.vector.tensor_mul(m3[:], b2[:], cc2_sb[:, tl])
                        nc.vector.tensor_mul(m4[:], a2[:], smp_sb[:, tl])
                        nc.vector.tensor_add(d0[0:64, tl],
                                             m1[0:64, :], m2[0:64, :])
                        nc.vector.tensor_add(d1[64:128, tl],
                                             m1[64:128, :], m2[64:128, :])
                        rb = rtmp.tile([128, 512], bf16, name="rb", tag="rtb",
                                       bufs=2)
                        nc.vector.tensor_add(rb[:], m3[:], m4[:])
                        nc.gpsimd.dma_start(d0[64:128, tl], rb[0:64, :])
                        nc.gpsimd.dma_start(d1[0:64, tl], rb[64:128, :])

                    def v_finish(b, tt, hl, pv):
                        vf = vfm_pool.tile([128, 512], bf16,
                                           name=f"vf_{b}_{tt}_{hl}", tag="vf")
                        nc.scalar.activation(vf[:], pv, Copy)
                        nc.sync.dma_start_transpose(
                            v_sb[(b, hl)][:, tt * 4:(tt + 1) * 4, :], vf[:])

                    def b_tile_wide(b, tt):
                        """chunk-major qkv tile: 6 psum groups at once
                        (pj x2 + ps halves + po x2)."""
                        pjA = psum.tile([128, 512], f32, name=f"bqA_{b}_{tt}",
                                        tag="pj")
                        pjB = psum.tile([128, 512], f32, name=f"bqB_{b}_{tt}",
                                        tag="pj")
                        phk = psum.tile([128, 2, 512], f32,
                                        name=f"bk_{b}_{tt}", tag="ps")
                        pv = [psum.tile([128, 512], f32, name=f"bv_{b}_{tt}_{hl}",
                                        tag="po") for hl in range(2)]
                        for ccs in range(cfg.NCC):
                            xb = xb_of(b, tt, ccs)
                            st = (ccs == 0)
                            sp = (ccs == cfg.NCC - 1)
                            nc.tensor.matmul(pjA[:], wqk_sb[:, ccs, 0:128],
                                             xb, start=st, stop=sp)
                            nc.tensor.matmul(pjB[:], wqk_sb[:, ccs, 128:256],
                                             xb, start=st, stop=sp)
                            nc.tensor.matmul(phk[:, 0, :],
                                             wqk_sb[:, ccs, 256:384],
                                             xb, start=st, stop=sp)
                            nc.tensor.matmul(phk[:, 1, :],
                                             wqk_sb[:, ccs, 384:512],
                                             xb, start=st, stop=sp)
                            for hl in range(2):
                                nc.tensor.matmul(
                                    pv[hl][:],
                                    wv_sb[:, ccs, hl * 128:(hl + 1) * 128],
                                    xb, start=st, stop=sp)
                        rope_pair(b, tt, pjA[:], pjB[:],
                                  qh_sb[(b, 0)], qh_sb[(b, 1)])
                        rope_pair(b, tt, phk[:, 0, :], phk[:, 1, :],
                                  kh_sb[(b, 0)], kh_sb[(b, 1)])
                        for hl in range(2):
                            v_finish(b, tt, hl, pv[hl][:])

                    def b_tile_units(b, tt):
                        """j-major qkv tile as a list of closures, each
                        holding at most 2 psum banks (interleavable with C)."""
                        units = [lambda b=b, tt=tt: fetch_x(b, tt)]
                        state = {}

                        # nparts>1 splits each 16-chunk chain into nparts
                        # emission subunits (same instructions, finer
                        # interleave granularity vs C blocks).
                        nparts = 4 if fine else 1
                        step = cfg.NCC // nparts

                        def jgroup(key, w_sb):
                            subs = []
                            for p0 in range(0, cfg.NCC, step):
                                def f(p0=p0, key=key, w_sb=w_sb):
                                    if p0 == 0:
                                        state[key] = psum.tile(
                                            [128, 512], f32,
                                            name=f"u{key}_{b}_{tt}",
                                            tag="pj")
                                    p = state[key][:]
                                    for ccs in range(p0, p0 + step):
                                        nc.tensor.matmul(
                                            p, w_sb(ccs), xb_of(b, tt, ccs),
                                            start=(ccs == 0),
                                            stop=(ccs == cfg.NCC - 1))
                                subs.append(f)
                            return subs

                        units.extend(jgroup("qA",
                                     lambda c: wqk_sb[:, c, 0:128]))
                        units.extend(jgroup("qB",
                                     lambda c: wqk_sb[:, c, 128:256]))
                        units.append(lambda: rope_pair(
                            b, tt, state["qA"][:], state["qB"][:],
                            qh_sb[(b, 0)], qh_sb[(b, 1)]))

                        units.extend(jgroup("kA",
                                     lambda c: wqk_sb[:, c, 256:384]))
                        units.extend(jgroup("kB",
                                     lambda c: wqk_sb[:, c, 384:512]))
                        units.append(lambda: rope_pair(
                            b, tt, state["kA"][:], state["kB"][:],
                            kh_sb[(b, 0)], kh_sb[(b, 1)]))

                        for hl in range(2):
                            for p0 in range(0, cfg.NCC, step):
                                def vv(hl=hl, p0=p0):
                                    key = f"v{hl}"
                                    if p0 == 0:
                                        state[key] = psum.tile(
                                            [128, 512], f32,
                                            name=f"uv_{b}_{tt}_{hl}",
                                            tag="pj")
                                    pvt = state[key]
                                    for ccs in range(p0, p0 + step):
                                        nc.tensor.matmul(
                                            pvt[:],
                                            wv_sb[:, ccs,
                                                  hl * 128:(hl + 1) * 128],
                                            xb_of(b, tt, ccs),
                                            start=(ccs == 0),
                                            stop=(ccs == cfg.NCC - 1))
                                    if p0 + step == cfg.NCC:
                                        v_finish(b, tt, hl, pvt[:])
                                units.append(vv)
                        return units

                    # ---------------- attention (phase C) -----------------
                    def make_c_blocks(b):
                        blocks = []
                        for tt in range(cfg.NT):
                            n_sc = (tt + 1) * 4
                            state = {}

                            def emit_po(entry, b=b, tt=tt, n_sc=n_sc,
                                        state=state):
                                sc_, off_, pr_ = entry
                                for h in range(2):
                                    nc.tensor.matmul(
                                        state["po"][h][:, off_:],
                                        v_sb[(b, h)][:, sc_, :],
                                        pr_[:, h, off_:],
                                        start=(sc_ == 0),
                                        stop=(sc_ == n_sc - 1))

                            def block(sc, b=b, tt=tt, n_sc=n_sc,
                                      state=state, emit_po=emit_po):
                                def f():
                                    if sc == 0:
                                        state["po"] = [
                                            psum.tile([128, 512], f32,
                                                      name=f"po_{b}_{tt}_{h}",
                                                      tag="po")
                                            for h in range(2)]
                                        state["ssum"] = ssum_pool.tile(
                                            [128, 2, 512], bf16,
                                            name=f"ss_{b}_{tt}", tag="ss")
                                        state["prq"] = []
                                    off = max(0, (sc - tt * 4) * 128)
                                    sl = slice(sc * 128, (sc + 1) * 128)
                                    qsl = slice(tt * 512 + off,
                                                (tt + 1) * 512)
                                    ph = psum.tile([128, 2, 512], f32,
                                                   name=f"ps_{b}_{tt}_{sc}",
                                                   tag="ps")
                                    for h in range(2):
                                        nc.tensor.matmul(
                                            ph[:, h, off:],
                                            kh_sb[(b, h)][:, sl],
                                            qh_sb[(b, h)][:, qsl],
                                            start=True, stop=True)
                                    pr = probs_pool.tile([128, 2, 512], bf16,
                                                         name="pr", tag="pr")
                                    nc.scalar.activation(pr[:, :, off:],
                                                         ph[:, :, off:], Exp,
                                                         scale=SCALE)
                                    if sc >= tt * 4:
                                        nc.vector.tensor_mul(
                                            pr[:, :, off:off + 128],
                                            pr[:, :, off:off + 128],
                                            masks_sb[:])
                                    ssum = state["ssum"]
                                    if sc == 0:
                                        nc.vector.tensor_copy(ssum[:], pr[:])
                                    else:
                                        nc.vector.tensor_add(
                                            ssum[:, :, off:],
                                            ssum[:, :, off:],
                                            pr[:, :, off:])
                                    state["prq"].append((sc, off, pr))
                                    if len(state["prq"]) > lag:
                                        emit_po(state["prq"].pop(0))
                                return f

                            def tail(b=b, tt=tt, state=state,
                                     emit_po=emit_po):
                                while state["prq"]:
                                    emit_po(state["prq"].pop(0))
                                pd = psum.tile([128, 2, 512], f32,
                                               name=f"pd_{b}_{tt}", tag="ps")
                                ssum = state["ssum"]
                                for h in range(2):
                                    nc.tensor.matmul(pd[:, h, :], ones_sb[:],
                                                     ssum[:, h, :],
                                                     start=True, stop=True)
                                for h in range(2):
                                    rec = rec_pool.tile([128, 512], f32,
                                                        name=f"rec_{h}",
                                                        tag="rec")
                                    nc.vector.reciprocal(rec[:], pd[:, h, :])
                                    nc.vector.tensor_mul(
                                        attn_sb[(b, h)][:,
                                                        tt * 512:(tt + 1) * 512],
                                        state["po"][h][:], rec[:])

                            blocks.extend(block(sc) for sc in range(n_sc))
                            blocks.append(tail)
                        return blocks

                    # ---------------- output projection (phase D) ----------
                    # wpe is batch-independent: cache 256-wide e-slices
                    # ([128,16,256] = 8KB each, 4 live). b1 runs e-order
                    # 4,5,6,7,0,1,2,3: its first four units hit still-live
                    # tiles and the lookahead prefetch hides the reloads.
                    WPE_BUFS = 4
                    NE8 = cfg.C // 256
                    wpe_tiles = {}
                    wpe_fifo = []
                    wpe_cnt = [0]

                    d_seq = ([(0, e) for e in range(NE8)] +
                             [(1, e) for e in
                              list(range(NE8 // 2, NE8)) +
                              list(range(NE8 // 2))])

                    def issue_wpe_e(e):
                        wpe_cnt[0] += 1
                        w = persist.tile([128, cfg.GRP, 256], bf16,
                                         name=f"wpe_{rep}_{wpe_cnt[0]}",
                                         tag="wpe", bufs=WPE_BUFS)
                        for g0 in range(0, cfg.GRP, 8):
                            nc.scalar.dma_start(
                                w[:, g0:g0 + 8, :],
                                wp_d[:, g0:g0 + 8, e * 256:(e + 1) * 256])
                        wpe_tiles[e] = w
                        wpe_fifo.append(e)
                        if len(wpe_fifo) > WPE_BUFS:
                            del wpe_tiles[wpe_fifo.pop(0)]

                    def wpe_prefetch(from_idx, lookahead=3):
                        """Issue tiles for upcoming pairs; never evict a tile
                        a not-yet-emitted pair still needs."""
                        protect = {e for (_bb, e) in d_seq[from_idx:]
                                   if e in wpe_tiles}
                        for j in range(from_idx,
                                       min(from_idx + lookahead, len(d_seq))):
                            e = d_seq[j][1]
                            if e in wpe_tiles:
                                continue
                            if (len(wpe_fifo) >= WPE_BUFS
                                    and wpe_fifo[0] in protect):
                                break
                            issue_wpe_e(e)
                            protect.add(e)

                    def d_unit_parts(b, e, hl, nparts=4):
                        """d_unit split into nparts emission subunits."""
                        dstate = {}
                        dstep = cfg.GRP // nparts
                        subs = []
                        for p0 in range(0, cfg.GRP, dstep):
                            def f(p0=p0):
                                if p0 == 0:
                                    if e not in wpe_tiles:
                                        issue_wpe_e(e)
                                    dstate["pp"] = psum.tile(
                                        [128, 256], f32,
                                        name=f"pp_{b}_{e}_{hl}", tag="pj")
                                    dstate["w"] = wpe_tiles[e]
                                pp, w = dstate["pp"], dstate["w"]
                                at = attn_sb[(b, hl)]
                                for u in range(p0, p0 + dstep):
                                    nc.tensor.matmul(
                                        pp[:], at[:, u::cfg.GRP], w[:, u, :],
                                        start=(u == 0),
                                        stop=(u == cfg.GRP - 1))
                                if p0 + dstep == cfg.GRP:
                                    el = slice(e * 256, (e + 1) * 256)
                                    stg = ostg_pool.tile(
                                        [128, 256], f32,
                                        name=f"stg_{b}_{e}_{hl}", tag="stg")
                                    (nc.gpsimd if stg_pool else
                                     nc.vector).tensor_copy(stg[:], pp[:])
                                    oq = (nc.sync if (e + hl) % 2 == 0
                                          else nc.gpsimd)
                                    oq.dma_start(out_d[rep, b, hl, :, el],
                                                 stg[:])
                                    wpe_prefetch(d_seq.index((b, e)) + 1)
                            subs.append(f)
                        return subs

                    def d_unit(b, e, hl, split=1):
                        def f():
                            idx = d_seq.index((b, e))
                            if e not in wpe_tiles:
                                issue_wpe_e(e)
                            w = wpe_tiles[e]
                            at = attn_sb[(b, hl)]
                            ew = 256 // split
                            for sp in range(split):
                                el = slice(e * 256 + sp * ew,
                                           e * 256 + (sp + 1) * ew)
                                wl = slice(sp * ew, (sp + 1) * ew)
                                pp = psum.tile([128, ew], f32,
                                               name=f"pp_{b}_{e}_{hl}_{sp}",
                                               tag="pj")
                                for u in range(cfg.GRP):
                                    nc.tensor.matmul(pp[:],
                                                     at[:, u::cfg.GRP],
                                                     w[:, u, wl],
                                                     start=(u == 0),
                                                     stop=(u == cfg.GRP - 1))
                                stg = ostg_pool.tile([128, ew], f32,
                                                     name=f"stg_{b}_{e}_{hl}",
                                                     tag="stg")
                                # DVE copy: ACT is exp-saturated in segment2,
                                # and a queued stg copy there delays the pj
                                # psum rotation (head-of-line).
                                (nc.gpsimd if stg_pool else
                                 nc.vector).tensor_copy(stg[:], pp[:])
                                oq = (nc.sync if (e + hl + sp) % 2 == 0
                                      else nc.gpsimd)
                                oq.dma_start(out_d[rep, b, hl, :, el],
                                             stg[:])
                            wpe_prefetch(idx + 1)
                        return f

                    def interleave(blocks, units, margin=4):
                        n = max(1, len(blocks) - margin)
                        per = len(units) / n
                        acc = 0.0
                        units = list(units)
                        for i, bl in enumerate(blocks):
                            bl()
                            if i < n:
                                acc += per
                            while acc >= 1.0 and units:
                                units.pop(0)()
                                acc -= 1.0
                        for u in units:
                            u()

                    # ------------------- emission order --------------------
                    first = (rep == 0) and loop_repeat is None

                    def dummy_out(src_tile):
                        stg = ostg_pool.tile([128, 512], f32,
                                             name="dstg", tag="stg")
                        nc.vector.tensor_copy(stg[:], src_tile)
                        nc.sync.dma_start(out_d[rep, 0, 0, :, 0:512], stg[:])

                    if "B" in phases:
                        fetch_x(0, 0, chunked=first)
                        fetch_x(0, 1)
                        b_tile_wide(0, 0)
                        fetch_x(0, 2)
                        if first:
                            # emitted here so their ACT-queue issue (and hence
                            # the transfers) trail tile-0's drain copies
                            # instead of stealing DMA bandwidth from its
                            # chunk train.
                            if cfg.T > 512:
                                nc.scalar.dma_start(cc2_sb[:, 512:cfg.T],
                                                    cc2_d[:, 512:cfg.T])
                                nc.scalar.dma_start(spm_sb[:, 512:cfg.T],
                                                    spm_d[:, 512:cfg.T])
                                nc.scalar.dma_start(smp_sb[:, 512:cfg.T],
                                                    smp_d[:, 512:cfg.T])
                            nc.scalar.dma_start(masks_sb[:], masks_d[:])
                        b_tile_wide(0, 1)
                        fetch_x(0, 3)
                        b_tile_wide(0, 2)
                        b_tile_wide(0, 3)
                    elif first:
                        nc.scalar.dma_start(masks_sb[:], masks_d[:])

                    if phases == "BCD":
                        wpe_prefetch(0, lookahead=2)
                        b1_units = []
                        for tt in range(cfg.NT):
                            b1_units.extend(b_tile_units(1, tt))
                        interleave(make_c_blocks(0), b1_units, margin=m0)

                        if fine:
                            d0_units = []
                            for e in range(NE8):
                                for hl in range(2):
                                    d0_units.extend(d_unit_parts(0, e, hl))
                        else:
                            d0_units = [d_unit(0, e, hl)
                                        for e in range(NE8)
                                        for hl in range(2)]
                        # margin=1: d0 fillers (pj tag only — no po conflict)
                        # run right through C(b1)'s tail, covering the
                        # attn-mul drain that gates D(b1)'s first LDW.
                        interleave(make_c_blocks(1), d0_units, margin=m1)

                        for _b1, e in d_seq[NE8:]:
                            for hl in range(2):
                                d_unit(1, e, hl)()
                    else:
                        if "B" in phases:
                            b1_units = []
                            for tt in range(cfg.NT):
                                b1_units.extend(b_tile_units(1, tt))
                            if "C" in phases:
                                interleave(make_c_blocks(0), b1_units,
                                           margin=6)
                            else:
                                for u in b1_units:
                                    u()
                        elif "C" in phases:
                            for bl in make_c_blocks(0):
                                bl()
                        if "C" in phases:
                            for bl in make_c_blocks(1):
                                bl()
                        if "D" in phases:
                            wpe_prefetch(0, lookahead=2)
                            for b in range(cfg.B):
                                for e in range(NE8):
                                    for hl in range(2):
                                        d_unit(b, e, hl)()
                        elif "C" in phases:
                            dummy_out(attn_sb[(1, 1)][:, 0:512])
                        else:
                            dummy_out(qh_sb[(1, 0)][:, 0:512])

            _loop_es.close()

    nc.compile()
    return nc


# =====================================================================
# Host-side input prep / output gather
# =====================================================================

def _part_major(a2d, ncc):
    """[ncc*128, F] -> [128, ncc, F] with row r = chunk*128 + p."""
    F = a2d.shape[1]
    return np.ascontiguousarray(
        a2d.reshape(ncc, 128, F).transpose(1, 0, 2))


def make_trig(cfg: Cfg):
    pos = np.arange(cfg.T, dtype=np.float64)[None, :]        # [1,T]
    j = np.arange(64, dtype=np.float64)[:, None]             # [64,1]
    inv = ROPE_BASE ** (-2.0 * j / Dh)
    ang = pos * inv                                          # [64,T]
    sin = np.sin(ang).astype(np.float32)
    cos = np.cos(ang).astype(np.float32)
    cc2 = np.concatenate([cos, cos], axis=0).astype(BF16)    # [128,T]
    spm = np.concatenate([-sin, sin], axis=0).astype(BF16)
    smp = np.concatenate([sin, -sin], axis=0).astype(BF16)
    return cc2, spm, smp


def make_masks():
    # one lower-triangular 128x128 block mask, duplicated for the 2 heads:
    # within a diagonal block the valid region is local col j >= partition p.
    p = np.arange(128)[:, None]
    jj = np.arange(128)[None, :]
    tri = (p <= jj)
    return np.stack([tri, tri], axis=1).astype(BF16)         # [128,2,128]


def make_in_maps(x, w_qkv, w_proj, cfg: Cfg = FULL, n_cores=N_CORES,
                 n_head=N_HEAD):
    x = np.asarray(x, np.float32)
    w_qkv = np.asarray(w_qkv, np.float32)
    w_proj = np.asarray(w_proj, np.float32)
    Cm = cfg.C

    xT = np.ascontiguousarray(x.reshape(cfg.BT, Cm).T)       # [C, BT]
    xt = _part_major(xT, cfg.NCC).astype(BF16)
    wp = _part_major(w_proj, cfg.GRP).astype(BF16)
    cc2, spm, smp = make_trig(cfg)
    masks = make_masks()

    wq = w_qkv[:, 0:Cm]
    wk = w_qkv[:, Cm:2 * Cm]
    wv_all = w_qkv[:, 2 * Cm:3 * Cm]

    in_maps = []
    for c in range(n_cores):
        h0, h1 = 2 * c, 2 * c + 1
        q0 = wq[:, h0 * 128:(h0 + 1) * 128]
        q1 = wq[:, h1 * 128:(h1 + 1) * 128]
        k0 = wk[:, h0 * 128:(h0 + 1) * 128]
        k1 = wk[:, h1 * 128:(h1 + 1) * 128]
        qA = np.concatenate([q0[:, 0:64], q1[:, 64:128]], axis=1)
        qB = np.concatenate([q0[:, 64:128], q1[:, 0:64]], axis=1)
        kA = np.concatenate([k0[:, 0:64], k1[:, 64:128]], axis=1)
        kB = np.concatenate([k0[:, 64:128], k1[:, 0:64]], axis=1)
        wqk = _part_major(
            np.concatenate([qA, qB, kA, kB], axis=1), cfg.NCC).astype(BF16)
        wv = _part_major(
            np.concatenate([wv_all[:, h0 * 128:(h0 + 1) * 128],
                            wv_all[:, h1 * 128:(h1 + 1) * 128]], axis=1),
            cfg.NCC).astype(BF16)
        in_maps.append(dict(xt=xt, wqk=wqk, wv=wv, wp=wp,
                            cc2=cc2, spm=spm, smp=smp, masks=masks))
    return in_maps


def gather(outs, cfg: Cfg = FULL):
    """outs: per-core [rep, B, H_LOCAL, 128, C] -> full [B, T, C]."""
    rows = np.concatenate(
        [o[-1].reshape(cfg.B, H_LOCAL * 128, cfg.C) for o in outs], axis=1)
    return np.ascontiguousarray(rows.reshape(cfg.B, cfg.T, cfg.C))


# =====================================================================
# Public entry point
# =====================================================================

_NC_CACHE = {}


def get_nc(debug=False):
    key = ("full", debug)
    if key not in _NC_CACHE:
        _NC_CACHE[key] = build_nc(FULL, debug=debug)
    return _NC_CACHE[key]


def kernel(x, w_qkv, w_proj):
    from concourse.bass_utils import run_bass_kernel_spmd
    nc = get_nc()
    in_maps = make_in_maps(x, w_qkv, w_proj)
    res = run_bass_kernel_spmd(nc, in_maps, list(range(N_CORES)))
    return gather([res.results[c]["out"] for c in range(N_CORES)])

